# revision 1
# baseline (speedup 1.0000x reference)
"""Trainium2 Bass kernel for the 2-layer GAT (nn_GAT_47459388621602).

Strategy (8 NeuronCores, SPMD, one NEFF; edge/graph-parallel per the
sharding hint — destination nodes sharded across cores, node-feature tables
replicated via AllGather):
  - Host: add self-loops, assign destination nodes to cores
    (degree-stratified, lo/hi-source-balanced), build per-core padded CSR
    gather index lists (int16, table split in two halves so indices fit
    int16 for dma_gather). Host prep is ~0.6s (list-based greedy balance +
    fully vectorized CSR fill), cached by an edge_index content hash.
  - Device, per core (fp16 tables/staging, f32 logit/softmax math):
      P1: fused [h1|ls1|ld1] = bn(x) @ W1[f] for the core's OWN 6272-node
          slab only (BN folded into weights on host; feature 129 via a
          second 1-partition accumulating matmul; attention projections
          a_src/a_dst folded into extra W1 columns). One whole-slab input
          DMA; one strided whole-slab table writeback.
      AllGather h1 slabs -> full fp16 table [50176, 256] (512B rows:
      [h1(128)|ls1(4)|ld1(4)|pad] — transfers under 512B pay a 2x DMA
      penalty, so the pad is free and ls1/ld1 ride along).
      P2: per 128-destination block: dma_gather source rows (<=1024 idxs
          per call — larger hangs the Q7 gather kernel), logits from
          gathered ls1 + local ld1 (+K-shift exp on ACT with per-head
          accumulated denominators), normalize attention weights FIRST
          (pn <= 1, fp16-safe), fp16 weighted sum via tree-reduction,
          +bias, ELU -> x2 slab kept transposed in SBUF (fp16).
      P3: fused [h2|ls2|ld2] slab from SBUF-resident x2 (one matmul per
          block, staged SBUF writeback in one DMA).
      AllGather h2 slabs -> full fp16 table [50176, 256].
      P4: layer-2 aggregation (ls2/ld2 ride in the gathered row) -> out
          slab [6272, 160] fp16, staged and written in one DMA.
  - Host re-assembles/unpermutes the 8 slabs into the full [50000, 160] f32.

Env knobs (measurement only; defaults are the production path):
  GAT_PHASES=1234, GAT_NBLK, GAT_REPEAT (loop the body inside one NEFF for
  repeat-amplified timing), GAT_GCHUNK=8.
"""
import hashlib
import os
import time

import numpy as np

import concourse.bacc as bacc
import concourse.mybir as mybir
import concourse.tile as tile
from concourse.bass_utils import run_bass_kernel_spmd
from concourse.library_config import mlp as mlp_library
from concourse.masks import make_identity

N_NODES = 50000
IN_F = 129
HID = 32
HEADS = 4
N_CLS = 40
NEG_SLOPE = 0.2
BN_EPS = 1e-5
NCORES = 8
BLK = 128
NBLK = 49
SLAB = NBLK * BLK           # 6272
NID = NCORES * SLAB         # 50176
HALF = NID // 2             # 25088
NEGM = -30000.0
K1 = 8.0
K2 = 12.0
F2 = 160                    # layer-2 message width
TAB1 = 256                  # fp16 layer-1 gather row (512B): [h1|ls1|ld1|pad]
TAB2 = 256                  # fp16 layer-2 gather row (512B): [h2|ls2|ld2|pad]
GCHUNK = int(os.environ.get("GAT_GCHUNK", "8"))  # w-cols (x128 idxs) per dma_gather
GRP = 4 * BLK

f32 = mybir.dt.float32
f16 = mybir.dt.float16
i16 = mybir.dt.int16


# ----------------------------------------------------------------- host prep
def _balance_greedy(src, dst):
    order = np.argsort(src, kind="stable")
    dst_by_src = dst[order]
    s_starts = np.searchsorted(src[order], np.arange(N_NODES))
    s_ends = np.searchsorted(src[order], np.arange(N_NODES) + 1)
    outdeg = s_ends - s_starts
    balance = [0] * N_NODES
    is_lo = np.zeros(N_NODES, dtype=bool)
    dbl = dst_by_src.tolist()
    cap = N_NODES // 2
    n_lo = n_hi = 0
    for n in np.argsort(-outdeg, kind="stable").tolist():
        s0, s1 = s_starts[n], s_ends[n]
        nb = dbl[s0:s1]
        tot = 0
        for d in nb:
            tot += balance[d]
        go_lo = tot <= 0
        if go_lo and n_lo >= cap:
            go_lo = False
        if (not go_lo) and n_hi >= cap:
            go_lo = True
        if go_lo:
            is_lo[n] = True
            n_lo += 1
            for d in nb:
                balance[d] += 1
        else:
            n_hi += 1
            for d in nb:
                balance[d] -= 1
    return is_lo


def _prep_indices(edge_index):
    src0 = np.asarray(edge_index[0], dtype=np.int64)
    dst0 = np.asarray(edge_index[1], dtype=np.int64)
    loops = np.arange(N_NODES, dtype=np.int64)
    src = np.concatenate([src0, loops])
    dst = np.concatenate([dst0, loops])
    deg = np.bincount(dst, minlength=N_NODES)

    is_lo_node = _balance_greedy(src, dst)
    is_lo_src = is_lo_node[src]
    deglo = np.bincount(dst[is_lo_src], minlength=N_NODES)
    deghi = deg - deglo

    # degree-stratified assignment; residue slot order keeps chunk types
    # aligned across cores so slot-wise max W is tight
    blocks = {}
    for half in range(2):
        ids = np.where(is_lo_node if half == 0 else ~is_lo_node)[0]
        ids = ids[np.argsort(-deg[ids], kind="stable")]
        n_strata = (len(ids) + GRP - 1) // GRP
        assert n_strata <= NBLK
        core_blocks = [[] for _ in range(4)]
        for s in range(n_strata):
            members = ids[s * GRP: min((s + 1) * GRP, len(ids))]
            m_sorted = members[np.argsort(-deglo[members], kind="stable")]
            chs = np.array_split(m_sorted, 4)
            for t, ch in enumerate(chs):
                core_blocks[(t - s) % 4].append((s, ch))
        for q in range(4):
            core_blocks[q].sort(key=lambda x: (x[0] // 4) * 4 + (x[0] + q) % 4)
            for b in range(NBLK):
                ch = core_blocks[q][b][1] if b < len(core_blocks[q]) else np.array([], dtype=np.int64)
                blk = ch[np.argsort(-deglo[ch], kind="stable")] if len(ch) else ch
                blocks[(half * 4 + q, b)] = blk

    node_cid = np.empty(N_NODES, dtype=np.int64)
    Wlo_qb = np.ones((NCORES, NBLK), dtype=np.int64)
    Whi_qb = np.ones((NCORES, NBLK), dtype=np.int64)
    for q in range(NCORES):
        for b in range(NBLK):
            blk = blocks[(q, b)]
            node_cid[blk] = q * SLAB + b * BLK + np.arange(len(blk))
            if len(blk):
                Wlo_qb[q, b] = max(1, int(deglo[blk].max()))
                Whi_qb[q, b] = max(1, int(deghi[blk].max()))
    Wlo = Wlo_qb.max(axis=0)
    Whi = Whi_qb.max(axis=0)
    S = int((Wlo + Whi).sum())
    offs = np.zeros(NBLK + 1, dtype=np.int64)
    offs[1:] = np.cumsum(Wlo + Whi)

    # vectorized padded-CSR fill
    E = len(src)
    src_cid = node_cid[src]
    dst_cid = node_cid[dst]
    eorder = np.argsort(dst_cid, kind="stable")
    sc = src_cid[eorder]
    dc = dst_cid[eorder]
    lo = is_lo_src[eorder]
    starts = np.searchsorted(dc, np.arange(NID))
    lo_cum = np.concatenate([[0], np.cumsum(lo)])
    pos = np.arange(E)
    seg_start = starts[dc]
    lo_rank = lo_cum[pos] - lo_cum[seg_start]
    hi_rank = (pos - seg_start) - lo_rank
    b_of = (dc // BLK) % NBLK
    q_of = dc // SLAB
    jj_of = dc % BLK
    col = offs[b_of] + np.where(lo, lo_rank, Wlo[b_of] + hi_rank)
    val = np.where(lo, sc, sc - HALF).astype(np.int16)
    flat = (q_of * BLK + jj_of) * S + col
    idx16 = np.zeros((NCORES, BLK, S), dtype=np.int16)
    idx16.ravel()[flat] = val
    maskflag = np.zeros((NCORES, BLK, S), dtype=np.bool_)
    maskflag.ravel()[flat] = True

    # compact wrapped idx stream [NCORES, 16, S*8]; device replicates to 128
    idxw = np.zeros((NCORES, 16, S * 8), dtype=np.int16)
    col0 = 0
    for b in range(NBLK):
        o = int(offs[b])
        for (w0, w1) in ((0, int(Wlo[b])), (int(Wlo[b]), int(Wlo[b] + Whi[b]))):
            nw = w1 - w0
            sl = idx16[:, :, o + w0:o + w1].transpose(0, 2, 1).reshape(NCORES, nw * BLK)
            idxw[:, :, col0:col0 + nw * 8] = \
                sl.reshape(NCORES, nw * 8, 16).transpose(0, 2, 1)
            col0 += nw * 8
    assert col0 == S * 8
    mneg = np.where(maskflag, np.float16(0.0), np.float16(NEGM))

    return dict(node_cid=node_cid, Wlo=Wlo.astype(int), Whi=Whi.astype(int),
                offs=offs, S=S, idxw=idxw, mneg=mneg)


# ----------------------------------------------------------------- program
def _build_program(Wlo, Whi, offs, S):
    PHASES = os.environ.get("GAT_PHASES", "1234")
    NB_RUN = int(os.environ.get("GAT_NBLK", str(NBLK)))
    REPEAT = int(os.environ.get("GAT_REPEAT", "1"))
    NQ = int(os.environ.get("GAT_NQUEUES", "4"))
    nc = bacc.Bacc("TRN2", target_bir_lowering=False, debug=False,
                   num_devices=NCORES, num_swdge_queues=NQ)
    gq = [0]

    # inputs
    t_xTs = nc.dram_tensor("xTs", [BLK, SLAB], f16, kind="ExternalInput")
    t_xls = nc.dram_tensor("xls", [1, SLAB], f16, kind="ExternalInput")
    t_W1 = nc.dram_tensor("W1f", [BLK, 136], f16, kind="ExternalInput")
    t_W1r = nc.dram_tensor("W1row", [1, 136], f16, kind="ExternalInput")
    t_W2 = nc.dram_tensor("W2f", [BLK, 168], f16, kind="ExternalInput")
    t_b1p = nc.dram_tensor("b1pb", [BLK, BLK], f32, kind="ExternalInput")
    t_csd = nc.dram_tensor("csdb", [BLK, HEADS], f32, kind="ExternalInput")
    t_idxw = nc.dram_tensor("idxw", [16, S * 8], i16, kind="ExternalInput")
    t_mneg = nc.dram_tensor("mneg", [BLK, S], f16, kind="ExternalInput")
    t_out = nc.dram_tensor("out2", [SLAB, F2], f16, kind="ExternalOutput")

    with tile.TileContext(nc) as tc:
        with (
            tc.tile_pool(name="const", bufs=1) as cpool,
            tc.tile_pool(name="dram", bufs=1, space="DRAM") as dpool,
        ):
            nc.gpsimd.load_library(mlp_library)

            # resident constants
            W1sb = cpool.tile([BLK, 136], f16)
            nc.sync.dma_start(out=W1sb[:], in_=t_W1[:])
            W1rsb = cpool.tile([1, 136], f16)
            nc.sync.dma_start(out=W1rsb[:], in_=t_W1r[:])
            W2sb = cpool.tile([BLK, 168], f16)
            nc.sync.dma_start(out=W2sb[:], in_=t_W2[:])
            b1pb = cpool.tile([BLK, BLK], f32)
            nc.sync.dma_start(out=b1pb[:], in_=t_b1p[:])
            csdb = cpool.tile([BLK, HEADS], f32)
            nc.sync.dma_start(out=csdb[:], in_=t_csd[:])
            idxw_sb = cpool.tile([BLK, S * 8], i16)
            for k in range(8):
                nc.sync.dma_start(out=idxw_sb[16 * k:16 * (k + 1), :],
                                  in_=t_idxw[:])
            mneg_sb = cpool.tile([BLK, S], f16)
            nc.sync.dma_start(out=mneg_sb[:], in_=t_mneg[:])
            ident = cpool.tile([BLK, BLK], f32)
            make_identity(nc, ident[:])
            kb1 = cpool.tile([BLK, 1], f32)
            nc.vector.memset(kb1[:], -K1)
            kb2 = cpool.tile([BLK, 1], f32)
            nc.vector.memset(kb2[:], -K2)
            # persistent per-slab state
            ld1sb = cpool.tile([BLK, NBLK * HEADS], f32)
            ld2sb = cpool.tile([BLK, NBLK * HEADS], f32)
            x2Tsb = cpool.tile([BLK, SLAB], f16)

            for _rep in range(REPEAT):
                # internal DRAM (per repetition: Shared tensors allow only a
                # single writing instruction)
                h1slab = dpool.tile([SLAB, TAB1], f16)
                h1full = dpool.tile([NID, TAB1], f16, addr_space="Shared")
                h2slab = dpool.tile([SLAB, TAB2], f16)
                h2full = dpool.tile([NID, TAB2], f16, addr_space="Shared")

                # ---------------- P1: own-slab h1 / ld1 (batched DMA)
                with (
                    tc.tile_pool(name="p1", bufs=1) as pool,
                    tc.tile_pool(name="p1ps", bufs=2, space="PSUM") as pspool,
                ):
                    if "1" in PHASES:
                        xTs_sb = pool.tile([BLK, SLAB], f16, tag="xTs")
                        nc.sync.dma_start(out=xTs_sb[:], in_=t_xTs[:])
                        xls_sb = pool.tile([1, SLAB], f16, tag="xls")
                        nc.sync.dma_start(out=xls_sb[:], in_=t_xls[:])
                        h1st = pool.tile([BLK, NBLK * 136], f16, tag="h1st")
                    for t in range(NBLK if "1" in PHASES else 0):
                        sl = slice(t * BLK, (t + 1) * BLK)
                        ps = pspool.tile([BLK, 136], f32)
                        nc.tensor.matmul(out=ps[:], lhsT=xTs_sb[:, sl], rhs=W1sb[:],
                                         start=True, stop=False)
                        nc.tensor.matmul(out=ps[:], lhsT=xls_sb[:, sl], rhs=W1rsb[:],
                                         start=False, stop=True)
                        nc.vector.tensor_copy(out=ld1sb[:, t * HEADS:(t + 1) * HEADS],
                                              in_=ps[:, 132:136])
                        nc.vector.tensor_copy(out=h1st[:, t * 136:(t + 1) * 136],
                                              in_=ps[:])
                    if "1" in PHASES:
                        nc.sync.dma_start(
                            out=h1slab[:, 0:136].rearrange("(t p) c -> p t c", p=BLK),
                            in_=h1st[:].rearrange("p (t c) -> p t c", c=136))

                # ---------------- AllGather h1 slabs
                if "2" in PHASES or "g" in PHASES:
                    nc.gpsimd.collective_compute(
                        "AllGather", mybir.AluOpType.bypass,
                        replica_groups=[list(range(NCORES))],
                        ins=[h1slab.opt()], outs=[h1full.opt()])

                # ---------------- P2: layer-1 aggregation -> x2Tsb (SBUF, fp16)
                with (
                    tc.tile_pool(name="p2g", bufs=3) as gpool,
                    tc.tile_pool(name="p2m", bufs=2) as mpool,
                    tc.tile_pool(name="p2s", bufs=3) as spool,
                    tc.tile_pool(name="p2ps", bufs=2, space="PSUM") as pspool,
                ):
                    for b in range(NB_RUN if "2" in PHASES else 0):
                        wl, wh = int(Wlo[b]), int(Whi[b])
                        wt = wl + wh
                        o = int(offs[b])
                        G = gpool.tile([BLK, wt * TAB1], f16, tag="G")
                        G3 = G[:].rearrange("p (w f) -> p w f", f=TAB1)
                        for (wbase, wlen, tab) in [(0, wl, h1full[0:HALF, :]),
                                                   (wl, wh, h1full[HALF:NID, :])]:
                            for w0 in range(0, wlen, GCHUNK):
                                wn = min(GCHUNK, wlen - w0)
                                nc.gpsimd.dma_gather(
                                    G3[:, wbase + w0:wbase + w0 + wn, :], tab,
                                    idxw_sb[:, (o + wbase + w0) * 8:(o + wbase + w0 + wn) * 8],
                                    wn * BLK, wn * BLK, TAB1,
                                    queue_num=gq[0] % NQ)
                                gq[0] += 1
                        ldc = spool.tile([BLK, HEADS], f32, tag="ldc")
                        nc.vector.tensor_tensor(
                            out=ldc[:], in0=ld1sb[:, b * HEADS:(b + 1) * HEADS],
                            in1=csdb[:], op=mybir.AluOpType.add)
                        M = mpool.tile([BLK, wt * BLK], f16, tag="M")
                        M4 = M[:].rearrange("p (w h c) -> p w h c", h=HEADS, c=HID)
                        G4 = G3[:, :, 0:BLK].rearrange("p w (h c) -> p w h c", c=HID)
                        lst = spool.tile([BLK, wt * HEADS], f32, tag="lst")
                        lst3 = lst[:].rearrange("p (w h) -> p w h", h=HEADS)
                        nc.vector.tensor_tensor(
                            out=lst3, in0=G3[:, :, BLK:BLK + HEADS],
                            in1=ldc[:].unsqueeze(1).to_broadcast([BLK, wt, HEADS]),
                            op=mybir.AluOpType.add)
                        nc.vector.tensor_tensor(
                            out=lst3, in0=lst3,
                            in1=mneg_sb[:, o:o + wt].unsqueeze(2).to_broadcast([BLK, wt, HEADS]),
                            op=mybir.AluOpType.add)
                        p_t = spool.tile([BLK, wt * HEADS], f32, tag="p")
                        p3 = p_t[:].rearrange("p (w h) -> p w h", h=HEADS)
                        nc.vector.tensor_scalar_mul(out=p_t[:], in0=lst[:], scalar1=NEG_SLOPE)
                        nc.vector.tensor_tensor(out=lst[:], in0=lst[:], in1=p_t[:],
                                                op=mybir.AluOpType.max)
                        den = spool.tile([BLK, HEADS], f32, tag="den")
                        for h in range(HEADS):
                            nc.scalar.activation(out=p3[:, :, h], in_=lst3[:, :, h],
                                                 func=mybir.ActivationFunctionType.Exp,
                                                 bias=kb1[:, 0:1],
                                                 accum_out=den[:, h:h + 1])
                        nc.vector.tensor_scalar_add(out=den[:], in0=den[:], scalar1=1e-30)
                        rcp = spool.tile([BLK, HEADS], f32, tag="rcp")
                        nc.vector.reciprocal(out=rcp[:], in_=den[:])
                        # normalize attention weights first (fp16-safe: pn <= 1)
                        p16 = spool.tile([BLK, wt * HEADS], f16, tag="p16")
                        p16_3 = p16[:].rearrange("p (w h) -> p w h", h=HEADS)
                        nc.vector.tensor_tensor(
                            out=p16_3, in0=p3,
                            in1=rcp[:].unsqueeze(1).to_broadcast([BLK, wt, HEADS]),
                            op=mybir.AluOpType.mult)
                        # M = G * pn (all-fp16: 2x DVE)
                        nc.vector.tensor_tensor(
                            out=M4, in0=G4,
                            in1=p16_3.unsqueeze(3).to_broadcast([BLK, wt, HEADS, HID]),
                            op=mybir.AluOpType.mult)
                        # tree reduce over w (fp16)
                        M3 = M[:].rearrange("p (w f) -> p w f", f=BLK)
                        w = wt
                        while w > 1:
                            hsz = w // 2
                            nc.vector.tensor_tensor(out=M3[:, 0:hsz, :], in0=M3[:, 0:hsz, :],
                                                    in1=M3[:, w - hsz:w, :],
                                                    op=mybir.AluOpType.add)
                            w -= hsz
                        x2 = spool.tile([BLK, BLK], f32, tag="x2")
                        nc.vector.tensor_tensor(out=x2[:], in0=M3[:, 0, :], in1=b1pb[:],
                                                op=mybir.AluOpType.add)
                        # elu
                        ex = spool.tile([BLK, BLK], f32, tag="ex")
                        nc.scalar.activation(out=ex[:], in_=x2[:],
                                             func=mybir.ActivationFunctionType.Exp)
                        nc.vector.tensor_scalar_add(out=ex[:], in0=ex[:], scalar1=-1.0)
                        nc.vector.tensor_scalar_min(out=ex[:], in0=ex[:], scalar1=0.0)
                        nc.vector.tensor_scalar_max(out=x2[:], in0=x2[:], scalar1=0.0)
                        nc.vector.tensor_tensor(out=x2[:], in0=x2[:], in1=ex[:],
                                                op=mybir.AluOpType.add)
                        # transpose -> persistent SBUF slab (fp16)
                        tps = pspool.tile([BLK, BLK], f32, tag="tps")
                        nc.tensor.transpose(out=tps[:], in_=x2[:], identity=ident[:])
                        nc.vector.tensor_copy(out=x2Tsb[:, b * BLK:(b + 1) * BLK],
                                              in_=tps[:])

                # ---------------- P3: own-slab h2 / ls2 / ld2 (batched DMA)
                with (
                    tc.tile_pool(name="p3", bufs=1) as pool,
                    tc.tile_pool(name="p3ps", bufs=2, space="PSUM") as pspool,
                ):
                    if "3" in PHASES:
                        h2st = pool.tile([BLK, NBLK * 168], f16, tag="h2st")
                    for t in range(NBLK if "3" in PHASES else 0):
                        ps = pspool.tile([BLK, 168], f32)
                        nc.tensor.matmul(out=ps[:],
                                         lhsT=x2Tsb[:, t * BLK:(t + 1) * BLK],
                                         rhs=W2sb[:], start=True, stop=True)
                        nc.vector.tensor_copy(out=ld2sb[:, t * HEADS:(t + 1) * HEADS],
                                              in_=ps[:, 164:168])
                        nc.vector.tensor_copy(out=h2st[:, t * 168:(t + 1) * 168],
                                              in_=ps[:])
                    if "3" in PHASES:
                        nc.sync.dma_start(
                            out=h2slab[:, 0:168].rearrange("(t p) c -> p t c", p=BLK),
                            in_=h2st[:].rearrange("p (t c) -> p t c", c=168))

                # ---------------- AllGather h2 slabs
                if "4" in PHASES or "h" in PHASES:
                    nc.gpsimd.collective_compute(
                        "AllGather", mybir.AluOpType.bypass,
                        replica_groups=[list(range(NCORES))],
                        ins=[h2slab.opt()], outs=[h2full.opt()])

                # ---------------- P4: layer-2 aggregation -> out2
                with (
                    tc.tile_pool(name="p4g", bufs=3) as gpool,
                    tc.tile_pool(name="p4m", bufs=2) as mpool,
                    tc.tile_pool(name="p4s", bufs=3) as spool,
                    tc.tile_pool(name="p4o", bufs=1) as opool,
                ):
                    if "4" in PHASES:
                        outst = opool.tile([BLK, NBLK * F2], f16, tag="outst")
                    for b in range(NB_RUN if "4" in PHASES else 0):
                        wl, wh = int(Wlo[b]), int(Whi[b])
                        wt = wl + wh
                        o = int(offs[b])
                        G = gpool.tile([BLK, wt * TAB2], f16, tag="G2")
                        G3 = G[:].rearrange("p (w f) -> p w f", f=TAB2)
                        for (wbase, wlen, tab) in [(0, wl, h2full[0:HALF, :]),
                                                   (wl, wh, h2full[HALF:NID, :])]:
                            for w0 in range(0, wlen, GCHUNK):
                                wn = min(GCHUNK, wlen - w0)
                                nc.gpsimd.dma_gather(
                                    G3[:, wbase + w0:wbase + w0 + wn, :], tab,
                                    idxw_sb[:, (o + wbase + w0) * 8:(o + wbase + w0 + wn) * 8],
                                    wn * BLK, wn * BLK, TAB2,
                                    queue_num=gq[0] % NQ)
                                gq[0] += 1
                        lst = spool.tile([BLK, wt * HEADS], f32, tag="lst2")
                        lst3 = lst[:].rearrange("p (w h) -> p w h", h=HEADS)
                        nc.vector.tensor_tensor(
                            out=lst3, in0=G3[:, :, F2:F2 + HEADS],
                            in1=ld2sb[:, b * HEADS:(b + 1) * HEADS]
                                .unsqueeze(1).to_broadcast([BLK, wt, HEADS]),
                            op=mybir.AluOpType.add)
                        nc.vector.tensor_tensor(
                            out=lst3, in0=lst3,
                            in1=mneg_sb[:, o:o + wt].unsqueeze(2).to_broadcast([BLK, wt, HEADS]),
                            op=mybir.AluOpType.add)
                        p_t = spool.tile([BLK, wt * HEADS], f32, tag="p2")
                        p3 = p_t[:].rearrange("p (w h) -> p w h", h=HEADS)
                        nc.vector.tensor_scalar_mul(out=p_t[:], in0=lst[:], scalar1=NEG_SLOPE)
                        nc.vector.tensor_tensor(out=lst[:], in0=lst[:], in1=p_t[:],
                                                op=mybir.AluOpType.max)
                        den = spool.tile([BLK, HEADS], f32, tag="den2")
                        for h in range(HEADS):
                            nc.scalar.activation(out=p3[:, :, h], in_=lst3[:, :, h],
                                                 func=mybir.ActivationFunctionType.Exp,
                                                 bias=kb2[:, 0:1],
                                                 accum_out=den[:, h:h + 1])
                        nc.vector.tensor_scalar_add(out=den[:], in0=den[:], scalar1=1e-30)
                        rcp = spool.tile([BLK, HEADS], f32, tag="rcp2")
                        nc.vector.reciprocal(out=rcp[:], in_=den[:])
                        # normalize attention weights first (fp16-safe: pn <= 1)
                        p16 = spool.tile([BLK, wt * HEADS], f16, tag="p16b")
                        p16_3 = p16[:].rearrange("p (w h) -> p w h", h=HEADS)
                        nc.vector.tensor_tensor(
                            out=p16_3, in0=p3,
                            in1=rcp[:].unsqueeze(1).to_broadcast([BLK, wt, HEADS]),
                            op=mybir.AluOpType.mult)
                        M = mpool.tile([BLK, wt * F2], f16, tag="M2")
                        M4 = M[:].rearrange("p (w h c) -> p w h c", h=HEADS, c=N_CLS)
                        G4 = G3[:, :, 0:F2].rearrange("p w (h c) -> p w h c", c=N_CLS)
                        nc.vector.tensor_tensor(
                            out=M4, in0=G4,
                            in1=p16_3.unsqueeze(3).to_broadcast([BLK, wt, HEADS, N_CLS]),
                            op=mybir.AluOpType.mult)
                        M3 = M[:].rearrange("p (w f) -> p w f", f=F2)
                        w = wt
                        while w > 1:
                            hsz = w // 2
                            nc.vector.tensor_tensor(out=M3[:, 0:hsz, :], in0=M3[:, 0:hsz, :],
                                                    in1=M3[:, w - hsz:w, :],
                                                    op=mybir.AluOpType.add)
                            w -= hsz
                        nc.vector.tensor_copy(out=outst[:, b * F2:(b + 1) * F2],
                                              in_=M3[:, 0, :])
                    if "4" in PHASES:
                        nc.sync.dma_start(
                            out=t_out[:].rearrange("(b p) c -> p b c", p=BLK),
                            in_=outst[:].rearrange("p (b c) -> p b c", c=F2))

    nc.compile()
    return nc


_CACHE = {}
_PREP_CACHE = {}


def _prep_cached(edge_index):
    ei = np.asarray(edge_index)
    key = (ei.shape, ei.dtype.str,
           hashlib.md5(np.ascontiguousarray(ei[:, ::101]).tobytes()).hexdigest())
    if key not in _PREP_CACHE:
        _PREP_CACHE[key] = _prep_indices(ei)
    return _PREP_CACHE[key]


def kernel(**inputs) -> np.ndarray:
    x = np.asarray(inputs["x"], np.float32)
    P = _prep_cached(inputs["edge_index"])
    node_cid = P["node_cid"]

    g = np.asarray(inputs["bn_gamma"], np.float32)
    bta = np.asarray(inputs["bn_beta"], np.float32)
    mu = np.asarray(inputs["bn_mean"], np.float32)
    var = np.asarray(inputs["bn_var"], np.float32)
    W1 = np.asarray(inputs["W1"], np.float32)
    a1s = np.asarray(inputs["a1_src"], np.float32)
    a1d = np.asarray(inputs["a1_dst"], np.float32)
    b1 = np.asarray(inputs["b1"], np.float32)
    W2 = np.asarray(inputs["W2"], np.float32)
    a2s = np.asarray(inputs["a2_src"], np.float32)
    a2d = np.asarray(inputs["a2_dst"], np.float32)

    s = g / np.sqrt(var + BN_EPS)
    W1p = (s[:, None] * W1).astype(np.float32)
    b1p = ((bta - mu * s) @ W1 + b1).astype(np.float32)
    A1s = np.zeros((HEADS * HID, HEADS), np.float32)
    A1d = np.zeros((HEADS * HID, HEADS), np.float32)
    A2s = np.zeros((HEADS * N_CLS, HEADS), np.float32)
    A2d = np.zeros((HEADS * N_CLS, HEADS), np.float32)
    for h in range(HEADS):
        A1s[h * HID:(h + 1) * HID, h] = a1s[h]
        A1d[h * HID:(h + 1) * HID, h] = a1d[h]
        A2s[h * N_CLS:(h + 1) * N_CLS, h] = a2s[h]
        A2d[h * N_CLS:(h + 1) * N_CLS, h] = a2d[h]
    W1f = np.concatenate([W1p, W1p @ A1s, W1p @ A1d], axis=1)  # [129, 136]
    csd = (b1p @ A1s + b1p @ A1d).astype(np.float32)
    W2f = np.concatenate([W2, W2 @ A2s, W2 @ A2d], axis=1).astype(np.float32)

    # x in cid space
    xp = np.zeros((NID, IN_F), np.float32)
    xp[node_cid] = x

    key = (tuple(P["Wlo"]), tuple(P["Whi"]),
           os.environ.get("GAT_PHASES", "1234"), os.environ.get("GAT_NBLK", ""),
           os.environ.get("GAT_REPEAT", ""), GCHUNK,
           os.environ.get("GAT_NQUEUES", ""))
    if key not in _CACHE:
        _CACHE[key] = _build_program(P["Wlo"], P["Whi"], P["offs"], P["S"])
    nc = _CACHE[key]

    common = {
        "W1f": np.ascontiguousarray(W1f[:BLK]).astype(np.float16),
        "W1row": W1f[BLK:BLK + 1].astype(np.float16),
        "W2f": W2f.astype(np.float16),
        "b1pb": np.tile(b1p[None, :], (BLK, 1)),
        "csdb": np.tile(csd[None, :], (BLK, 1)),
    }

    in_maps = []
    for q in range(NCORES):
        m = dict(common)
        qs = slice(q * SLAB, (q + 1) * SLAB)
        m["xTs"] = np.ascontiguousarray(xp[qs, :BLK].T).astype(np.float16)
        m["xls"] = np.ascontiguousarray(xp[qs, BLK:BLK + 1].T).astype(np.float16)
        m["idxw"] = np.ascontiguousarray(P["idxw"][q])
        m["mneg"] = np.ascontiguousarray(P["mneg"][q])
        in_maps.append(m)

    t0 = time.time()
    res = run_bass_kernel_spmd(nc, in_maps, core_ids=list(range(NCORES)))
    global last_run_seconds
    last_run_seconds = time.time() - t0
    outfull = np.concatenate([r["out2"] for r in res.results], axis=0)
    return outfull[node_cid].astype(np.float32)


last_run_seconds = None



# revision 20
# speedup vs baseline: 1.1225x; 1.1225x over previous
"""Trainium2 Bass kernel for the 2-layer GAT (nn_GAT_47459388621602).

Strategy (8 NeuronCores, SPMD, one NEFF; edge/graph-parallel per the
sharding hint — destination nodes sharded across cores, node-feature tables
replicated via AllGather):
  - Host: add self-loops, assign destination nodes to cores
    (degree-stratified, lo/hi-source-balanced), build per-core padded CSR
    gather index lists (int16, table split in two halves so indices fit
    int16 for dma_gather). Host prep is ~0.6s (list-based greedy balance +
    fully vectorized CSR fill), cached by an edge_index content hash.
  - Device, per core (fp16 tables/staging, f32 logit/softmax math):
      P1: fused [h1|ls1|ld1] = bn(x) @ W1[f] for the core's OWN 6272-node
          slab only (BN folded into weights on host; feature 129 via a
          second 1-partition accumulating matmul; attention projections
          a_src/a_dst folded into extra W1 columns). One whole-slab input
          DMA; one strided whole-slab table writeback.
      AllGather h1 slabs -> full fp16 table [50176, 256] (512B rows:
      [h1(128)|ls1(4)|ld1(4)|pad] — transfers under 512B pay a 2x DMA
      penalty, so the pad is free and ls1/ld1 ride along).
      P2: per 128-destination block: dma_gather source rows (<=1024 idxs
          per call — larger hangs the Q7 gather kernel), logits from
          gathered ls1 + local ld1 (+K-shift exp on ACT with per-head
          accumulated denominators), normalize attention weights FIRST
          (pn <= 1, fp16-safe), fp16 weighted sum via tree-reduction,
          +bias, ELU -> x2 slab kept transposed in SBUF (fp16).
      P3: fused [h2|ls2|ld2] slab from SBUF-resident x2 (one matmul per
          block, staged SBUF writeback in one DMA).
      AllGather h2 slabs -> full fp16 table [50176, 256].
      P4: layer-2 aggregation (ls2/ld2 ride in the gathered row) -> out
          slab [6272, 160] fp16, staged and written in one DMA.
  - Host re-assembles/unpermutes the 8 slabs into the full [50000, 160] f32.

Env knobs (measurement only; defaults are the production path):
  GAT_PHASES=1234, GAT_NBLK, GAT_REPEAT (loop the body inside one NEFF for
  repeat-amplified timing), GAT_GCHUNK=8.
"""
import hashlib
import os
import time

import numpy as np

import concourse.bacc as bacc
import concourse.mybir as mybir
import concourse.tile as tile
from concourse.bass_utils import run_bass_kernel_spmd
from concourse.library_config import mlp as mlp_library
from concourse.masks import make_identity

N_NODES = 50000
IN_F = 129
HID = 32
HEADS = 4
N_CLS = 40
NEG_SLOPE = 0.2
BN_EPS = 1e-5
NCORES = 8
BLK = 128
NBLK = 49
SLAB = NBLK * BLK           # 6272
NID = NCORES * SLAB         # 50176
HALF = NID // 2             # 25088
NEGM = -30000.0
K1 = 8.0
K2 = 12.0
F2 = 160                    # layer-2 message width
TAB1 = 256                  # fp16 layer-1 gather row (512B): [h1|ls1|ld1|pad]
TAB2 = 256                  # fp16 layer-2 gather row (512B): [h2|ls2|ld2|pad]
GCHUNK = int(os.environ.get("GAT_GCHUNK", "8"))  # w-cols (x128 idxs) per dma_gather
GRP = 4 * BLK

f32 = mybir.dt.float32
f16 = mybir.dt.float16
i16 = mybir.dt.int16


# ----------------------------------------------------------------- host prep
def _balance_greedy(src, dst):
    order = np.argsort(src, kind="stable")
    dst_by_src = dst[order]
    s_starts = np.searchsorted(src[order], np.arange(N_NODES))
    s_ends = np.searchsorted(src[order], np.arange(N_NODES) + 1)
    outdeg = s_ends - s_starts
    balance = [0] * N_NODES
    is_lo = np.zeros(N_NODES, dtype=bool)
    dbl = dst_by_src.tolist()
    cap = N_NODES // 2
    n_lo = n_hi = 0
    for n in np.argsort(-outdeg, kind="stable").tolist():
        s0, s1 = s_starts[n], s_ends[n]
        nb = dbl[s0:s1]
        tot = 0
        for d in nb:
            tot += balance[d]
        go_lo = tot <= 0
        if go_lo and n_lo >= cap:
            go_lo = False
        if (not go_lo) and n_hi >= cap:
            go_lo = True
        if go_lo:
            is_lo[n] = True
            n_lo += 1
            for d in nb:
                balance[d] += 1
        else:
            n_hi += 1
            for d in nb:
                balance[d] -= 1
    return is_lo


def _prep_indices(edge_index):
    src0 = np.asarray(edge_index[0], dtype=np.int64)
    dst0 = np.asarray(edge_index[1], dtype=np.int64)
    loops = np.arange(N_NODES, dtype=np.int64)
    src = np.concatenate([src0, loops])
    dst = np.concatenate([dst0, loops])
    deg = np.bincount(dst, minlength=N_NODES)

    is_lo_node = _balance_greedy(src, dst)
    is_lo_src = is_lo_node[src]
    deglo = np.bincount(dst[is_lo_src], minlength=N_NODES)
    deghi = deg - deglo

    # degree-stratified assignment; residue slot order keeps chunk types
    # aligned across cores so slot-wise max W is tight
    blocks = {}
    for half in range(2):
        ids = np.where(is_lo_node if half == 0 else ~is_lo_node)[0]
        ids = ids[np.argsort(-deg[ids], kind="stable")]
        n_strata = (len(ids) + GRP - 1) // GRP
        assert n_strata <= NBLK
        core_blocks = [[] for _ in range(4)]
        for s in range(n_strata):
            members = ids[s * GRP: min((s + 1) * GRP, len(ids))]
            m_sorted = members[np.argsort(-deglo[members], kind="stable")]
            chs = np.array_split(m_sorted, 4)
            for t, ch in enumerate(chs):
                core_blocks[(t - s) % 4].append((s, ch))
        for q in range(4):
            core_blocks[q].sort(key=lambda x: (x[0] // 4) * 4 + (x[0] + q) % 4)
            for b in range(NBLK):
                ch = core_blocks[q][b][1] if b < len(core_blocks[q]) else np.array([], dtype=np.int64)
                blk = ch[np.argsort(-deglo[ch], kind="stable")] if len(ch) else ch
                blocks[(half * 4 + q, b)] = blk

    node_cid = np.empty(N_NODES, dtype=np.int64)
    Wlo_qb = np.ones((NCORES, NBLK), dtype=np.int64)
    Whi_qb = np.ones((NCORES, NBLK), dtype=np.int64)
    for q in range(NCORES):
        for b in range(NBLK):
            blk = blocks[(q, b)]
            node_cid[blk] = q * SLAB + b * BLK + np.arange(len(blk))
            if len(blk):
                Wlo_qb[q, b] = max(1, int(deglo[blk].max()))
                Whi_qb[q, b] = max(1, int(deghi[blk].max()))
    Wlo = Wlo_qb.max(axis=0)
    Whi = Whi_qb.max(axis=0)
    S = int((Wlo + Whi).sum())
    offs = np.zeros(NBLK + 1, dtype=np.int64)
    offs[1:] = np.cumsum(Wlo + Whi)

    # pad slots gather a "hole" row (an unused cid) whose ls columns are
    # force-written to NEGM on device, so exp()->0 without any mask op
    holes = np.setdiff1d(np.arange(NID), node_cid)
    hole_lo = int(holes[holes < HALF][0])
    hole_hi = int(holes[holes >= HALF][0])

    # vectorized padded-CSR fill
    E = len(src)
    src_cid = node_cid[src]
    dst_cid = node_cid[dst]
    eorder = np.argsort(dst_cid, kind="stable")
    sc = src_cid[eorder]
    dc = dst_cid[eorder]
    lo = is_lo_src[eorder]
    starts = np.searchsorted(dc, np.arange(NID))
    lo_cum = np.concatenate([[0], np.cumsum(lo)])
    pos = np.arange(E)
    seg_start = starts[dc]
    lo_rank = lo_cum[pos] - lo_cum[seg_start]
    hi_rank = (pos - seg_start) - lo_rank
    b_of = (dc // BLK) % NBLK
    q_of = dc // SLAB
    jj_of = dc % BLK
    col = offs[b_of] + np.where(lo, lo_rank, Wlo[b_of] + hi_rank)
    val = np.where(lo, sc, sc - HALF).astype(np.int16)
    flat = (q_of * BLK + jj_of) * S + col
    colhi = np.zeros(S, dtype=bool)
    for b in range(NBLK):
        colhi[offs[b] + Wlo[b]:offs[b + 1]] = True
    idx16 = np.where(colhi, np.int16(hole_hi - HALF),
                     np.int16(hole_lo))[None, None, :].repeat(
                         NCORES, 0).repeat(BLK, 1).astype(np.int16)
    idx16.ravel()[flat] = val
    # per-(core, block) fill levels -> hole-row mask [NCORES, BLK, NBLK]
    lens = np.zeros((NCORES, NBLK), dtype=np.int64)
    for q in range(NCORES):
        for b in range(NBLK):
            lens[q, b] = len(blocks[(q, b)])
    jj = np.arange(BLK)
    nmask = np.where(jj[None, :, None] < lens[:, None, :],
                     np.float16(0.0), np.float16(NEGM))  # [NCORES, BLK, NBLK]
    nmask = np.repeat(nmask[:, :, :, None], HEADS, axis=3).reshape(
        NCORES, BLK, NBLK * HEADS)

    # compact wrapped idx stream [NCORES, 16, S*8]; device replicates to 128
    idxw = np.zeros((NCORES, 16, S * 8), dtype=np.int16)
    col0 = 0
    for b in range(NBLK):
        o = int(offs[b])
        for (w0, w1) in ((0, int(Wlo[b])), (int(Wlo[b]), int(Wlo[b] + Whi[b]))):
            nw = w1 - w0
            sl = idx16[:, :, o + w0:o + w1].transpose(0, 2, 1).reshape(NCORES, nw * BLK)
            idxw[:, :, col0:col0 + nw * 8] = \
                sl.reshape(NCORES, nw * 8, 16).transpose(0, 2, 1)
            col0 += nw * 8
    assert col0 == S * 8

    return dict(node_cid=node_cid, Wlo=Wlo.astype(int), Whi=Whi.astype(int),
                offs=offs, S=S, idxw=idxw, nmask=nmask)


# ----------------------------------------------------------------- program
def _build_program(Wlo, Whi, offs, S):
    PHASES = os.environ.get("GAT_PHASES", "1234")
    NB_RUN = int(os.environ.get("GAT_NBLK", str(NBLK)))
    REPEAT = int(os.environ.get("GAT_REPEAT", "1"))
    NQ = int(os.environ.get("GAT_NQUEUES", "4"))
    nc = bacc.Bacc("TRN2", target_bir_lowering=False, debug=False,
                   num_devices=NCORES, num_swdge_queues=NQ)
    gq = [0]

    # inputs
    t_xTs = nc.dram_tensor("xTs", [BLK, SLAB], f16, kind="ExternalInput")
    t_xls = nc.dram_tensor("xls", [1, SLAB], f16, kind="ExternalInput")
    t_W1 = nc.dram_tensor("W1f", [BLK, 136], f16, kind="ExternalInput")
    t_W1r = nc.dram_tensor("W1row", [1, 136], f16, kind="ExternalInput")
    t_W2 = nc.dram_tensor("W2f", [BLK, 168], f16, kind="ExternalInput")
    t_b1p = nc.dram_tensor("b1pb", [BLK, BLK], f32, kind="ExternalInput")
    t_csd = nc.dram_tensor("csdb", [BLK, HEADS], f32, kind="ExternalInput")
    t_idxw = nc.dram_tensor("idxw", [16, S * 8], i16, kind="ExternalInput")
    t_nmask = nc.dram_tensor("nmask", [BLK, NBLK * HEADS], f16,
                             kind="ExternalInput")
    t_out = nc.dram_tensor("out2", [SLAB, F2], f16, kind="ExternalOutput")

    with tile.TileContext(nc) as tc:
        with (
            tc.tile_pool(name="const", bufs=1) as cpool,
            tc.tile_pool(name="dram", bufs=1, space="DRAM") as dpool,
        ):
            nc.gpsimd.load_library(mlp_library)

            # resident constants
            W1sb = cpool.tile([BLK, 136], f16)
            nc.sync.dma_start(out=W1sb[:], in_=t_W1[:])
            W1rsb = cpool.tile([1, 136], f16)
            nc.sync.dma_start(out=W1rsb[:], in_=t_W1r[:])
            W2sb = cpool.tile([BLK, 168], f16)
            nc.sync.dma_start(out=W2sb[:], in_=t_W2[:])
            b1pb = cpool.tile([BLK, BLK], f32)
            nc.sync.dma_start(out=b1pb[:], in_=t_b1p[:])
            csdb = cpool.tile([BLK, HEADS], f32)
            nc.sync.dma_start(out=csdb[:], in_=t_csd[:])
            idxw_sb = cpool.tile([BLK, S * 8], i16)
            for k in range(8):
                nc.sync.dma_start(out=idxw_sb[16 * k:16 * (k + 1), :],
                                  in_=t_idxw[:])
            nmask_sb = cpool.tile([BLK, NBLK * HEADS], f16)
            nc.sync.dma_start(out=nmask_sb[:], in_=t_nmask[:])
            ident = cpool.tile([BLK, BLK], f32)
            make_identity(nc, ident[:])
            kb1 = cpool.tile([BLK, 1], f32)
            nc.vector.memset(kb1[:], -K1)
            kb2 = cpool.tile([BLK, 1], f32)
            nc.vector.memset(kb2[:], -K2)
            zeros = cpool.tile([BLK, BLK], f32)
            nc.vector.memset(zeros[:], 0.0)
            # persistent per-slab state
            ld1sb = cpool.tile([BLK, NBLK * HEADS], f32)
            ld2sb = cpool.tile([BLK, NBLK * HEADS], f32)
            x2Tsb = cpool.tile([BLK, SLAB], f16)
            # diagnostic compute-only modes: resident garbage gather tile
            wtmax = int(max(Wlo[b] + Whi[b] for b in range(NBLK)))
            if "6" in PHASES and "2" not in PHASES:
                Ggarb = cpool.tile([BLK, wtmax * TAB1], f16)
                nc.vector.memset(Ggarb[:], 0.5)
            if "8" in PHASES and "4" not in PHASES:
                Ggarb2 = cpool.tile([BLK, wtmax * TAB2], f16)
                nc.vector.memset(Ggarb2[:], 0.5)

            for _rep in range(REPEAT):
                # internal DRAM (per repetition: Shared tensors allow only a
                # single writing instruction)
                h1slab = dpool.tile([SLAB, TAB1], f16)
                h1full = dpool.tile([NID, TAB1], f16, addr_space="Shared")
                h2slab = dpool.tile([SLAB, TAB2], f16)
                h2full = dpool.tile([NID, TAB2], f16, addr_space="Shared")

                # ---------------- P1: own-slab h1 / ld1 (batched DMA)
                with (
                    tc.tile_pool(name="p1", bufs=1) as pool,
                    tc.tile_pool(name="p1ps", bufs=2, space="PSUM") as pspool,
                ):
                    if "1" in PHASES:
                        xTs_sb = pool.tile([BLK, SLAB], f16, tag="xTs")
                        nc.sync.dma_start(out=xTs_sb[:], in_=t_xTs[:])
                        xls_sb = pool.tile([1, SLAB], f16, tag="xls")
                        nc.sync.dma_start(out=xls_sb[:], in_=t_xls[:])
                        h1st = pool.tile([BLK, NBLK * 136], f16, tag="h1st")
                    for t in range(NBLK if "1" in PHASES else 0):
                        sl = slice(t * BLK, (t + 1) * BLK)
                        ps = pspool.tile([BLK, 136], f32)
                        nc.tensor.matmul(out=ps[:], lhsT=xTs_sb[:, sl], rhs=W1sb[:],
                                         start=True, stop=False)
                        nc.tensor.matmul(out=ps[:], lhsT=xls_sb[:, sl], rhs=W1rsb[:],
                                         start=False, stop=True)
                        nc.vector.tensor_copy(out=ld1sb[:, t * HEADS:(t + 1) * HEADS],
                                              in_=ps[:, 132:136])
                        nc.vector.tensor_copy(out=h1st[:, t * 136:(t + 1) * 136],
                                              in_=ps[:])
                    if "1" in PHASES:
                        # hole rows: force ls columns to NEGM so any gather of
                        # them yields exp()->0 (replaces the per-block mask op)
                        h1ls = h1st[:].rearrange("p (t c) -> p t c", c=136)
                        nc.vector.tensor_tensor(
                            out=h1ls[:, :, 128:132], in0=h1ls[:, :, 128:132],
                            in1=nmask_sb[:].rearrange("p (t h) -> p t h", h=HEADS),
                            op=mybir.AluOpType.add)
                        nc.sync.dma_start(
                            out=h1slab[:, 0:136].rearrange("(t p) c -> p t c", p=BLK),
                            in_=h1st[:].rearrange("p (t c) -> p t c", c=136))

                # ---------------- AllGather h1 slabs
                if "2" in PHASES or "g" in PHASES:
                    nc.gpsimd.collective_compute(
                        "AllGather", mybir.AluOpType.bypass,
                        replica_groups=[list(range(NCORES))],
                        ins=[h1slab.opt()], outs=[h1full.opt()])

                # ---------------- P2: layer-1 aggregation -> x2Tsb (SBUF, fp16)
                p2_gather = "2" in PHASES or "5" in PHASES
                p2_comp = "2" in PHASES or "6" in PHASES
                with (
                    tc.tile_pool(name="p2g", bufs=3) as gpool,
                    tc.tile_pool(name="p2m", bufs=2) as mpool,
                    tc.tile_pool(name="p2s", bufs=3) as spool,
                    tc.tile_pool(name="p2h", bufs=1) as h2pool,
                    tc.tile_pool(name="p2ps", bufs=2, space="PSUM") as pspool,
                ):
                    if "3" in PHASES:
                        h2st = h2pool.tile([BLK, NBLK * 168], f16, tag="h2st")
                    for b in range(NB_RUN if (p2_gather or p2_comp) else 0):
                        wl, wh = int(Wlo[b]), int(Whi[b])
                        wt = wl + wh
                        o = int(offs[b])
                        if p2_gather:
                            # diag knob: gather a smaller slice of each row
                            # (bytes/descriptor probe; only for gather-only mode)
                            TABd = int(os.environ.get("GAT_TABD", "0"))
                            TABg = TABd if (TABd and "2" not in PHASES) else TAB1
                            G = gpool.tile([BLK, wt * TABg], f16, tag="G")
                            G3 = G[:].rearrange("p (w f) -> p w f", f=TABg)
                            for (wbase, wlen, tab) in [(0, wl, h1full[0:HALF, 0:TABg]),
                                                       (wl, wh, h1full[HALF:NID, 0:TABg])]:
                                for w0 in range(0, wlen, GCHUNK):
                                    wn = min(GCHUNK, wlen - w0)
                                    nc.gpsimd.dma_gather(
                                        G3[:, wbase + w0:wbase + w0 + wn, :], tab,
                                        idxw_sb[:, (o + wbase + w0) * 8:(o + wbase + w0 + wn) * 8],
                                        wn * BLK, wn * BLK, TABg,
                                        elem_step=TAB1,
                                        queue_num=gq[0] % NQ)
                                    gq[0] += 1
                        if not p2_comp:
                            continue
                        if not ("2" in PHASES):
                            G3 = Ggarb[:, 0:wt * TAB1].rearrange(
                                "p (w f) -> p w f", f=TAB1)
                        ldc = spool.tile([BLK, HEADS], f32, tag="ldc")
                        nc.vector.tensor_tensor(
                            out=ldc[:], in0=ld1sb[:, b * HEADS:(b + 1) * HEADS],
                            in1=csdb[:], op=mybir.AluOpType.add)
                        M = mpool.tile([BLK, wt * BLK], f16, tag="M")
                        lst = spool.tile([BLK, wt * HEADS], f32, tag="lst")
                        lst3 = lst[:].rearrange("p (w h) -> p w h", h=HEADS)
                        nc.vector.tensor_tensor(
                            out=lst3, in0=G3[:, :, BLK:BLK + HEADS],
                            in1=ldc[:].unsqueeze(1).to_broadcast([BLK, wt, HEADS]),
                            op=mybir.AluOpType.add)
                        # leaky relu in one pass: max(x, slope*x)
                        nc.vector.scalar_tensor_tensor(
                            out=lst[:], in0=lst[:], scalar=NEG_SLOPE, in1=lst[:],
                            op0=mybir.AluOpType.mult, op1=mybir.AluOpType.max)
                        # exp (K-shifted) directly to fp16, accumulate denoms
                        p16u = spool.tile([BLK, wt * HEADS], f16, tag="p16u")
                        p16u3 = p16u[:].rearrange("p (w h) -> p w h", h=HEADS)
                        den = spool.tile([BLK, HEADS], f32, tag="den")
                        for h in range(HEADS):
                            nc.scalar.activation(out=p16u3[:, :, h], in_=lst3[:, :, h],
                                                 func=mybir.ActivationFunctionType.Exp,
                                                 bias=kb1[:, 0:1],
                                                 accum_out=den[:, h:h + 1])
                        nc.vector.tensor_scalar_add(out=den[:], in0=den[:], scalar1=1e-30)
                        rcp = spool.tile([BLK, HEADS], f32, tag="rcp")
                        nc.vector.reciprocal(out=rcp[:], in_=den[:])
                        # normalize + duplicate each weight (pairs enable 2x TT)
                        pn2 = spool.tile([BLK, wt * HEADS * 2], f16, tag="pn2")
                        nc.vector.tensor_tensor(
                            out=pn2[:].rearrange("p (w h two) -> p w h two",
                                                 h=HEADS, two=2),
                            in0=p16u3.unsqueeze(3).to_broadcast([BLK, wt, HEADS, 2]),
                            in1=rcp[:].unsqueeze(1).unsqueeze(3)
                                .to_broadcast([BLK, wt, HEADS, 2]),
                            op=mybir.AluOpType.mult)
                        # M = G * pn (paired views keep innermost stride-1: 2x DVE)
                        C2 = HID // 2
                        nc.vector.tensor_tensor(
                            out=M[:].rearrange("p (w h c2 two) -> p w h c2 two",
                                               h=HEADS, c2=C2, two=2),
                            in0=G3[:, :, 0:BLK].rearrange(
                                "p w (h c2 two) -> p w h c2 two", c2=C2, two=2),
                            in1=pn2[:].rearrange("p (w h two) -> p w h two",
                                                 h=HEADS, two=2)
                                .unsqueeze(3).to_broadcast([BLK, wt, HEADS, C2, 2]),
                            op=mybir.AluOpType.mult)
                        # tree reduce over w (fp16)
                        M3 = M[:].rearrange("p (w f) -> p w f", f=BLK)
                        w = wt
                        while w > 1:
                            hsz = w // 2
                            nc.vector.tensor_tensor(out=M3[:, 0:hsz, :], in0=M3[:, 0:hsz, :],
                                                    in1=M3[:, w - hsz:w, :],
                                                    op=mybir.AluOpType.add)
                            w -= hsz
                        x2 = spool.tile([BLK, BLK], f32, tag="x2")
                        nc.vector.tensor_tensor(out=x2[:], in0=M3[:, 0, :], in1=b1pb[:],
                                                op=mybir.AluOpType.add)
                        # elu: x2 = max(x2,0) + min(exp(x2)-1, 0)  (1 ACT + 2 fused DVE)
                        ex = spool.tile([BLK, BLK], f32, tag="ex")
                        nc.scalar.activation(out=ex[:], in_=x2[:],
                                             func=mybir.ActivationFunctionType.Exp)
                        nc.vector.scalar_tensor_tensor(
                            out=ex[:], in0=ex[:], scalar=-1.0, in1=zeros[:],
                            op0=mybir.AluOpType.add, op1=mybir.AluOpType.min)
                        nc.vector.scalar_tensor_tensor(
                            out=x2[:], in0=x2[:], scalar=0.0, in1=ex[:],
                            op0=mybir.AluOpType.max, op1=mybir.AluOpType.add)
                        # transpose -> persistent SBUF slab (fp16)
                        tps = pspool.tile([BLK, BLK], f32, tag="tps")
                        nc.tensor.transpose(out=tps[:], in_=x2[:], identity=ident[:])
                        nc.vector.tensor_copy(out=x2Tsb[:, b * BLK:(b + 1) * BLK],
                                              in_=tps[:])
                        # fused P3: layer-2 projection for this block (PE + ACT)
                        if "3" in PHASES:
                            ps2 = pspool.tile([BLK, 168], f32, tag="mm2")
                            nc.tensor.matmul(
                                out=ps2[:], lhsT=x2Tsb[:, b * BLK:(b + 1) * BLK],
                                rhs=W2sb[:], start=True, stop=True)
                            nc.vector.tensor_copy(
                                out=ld2sb[:, b * HEADS:(b + 1) * HEADS],
                                in_=ps2[:, 164:168])
                            nc.scalar.activation(
                                out=h2st[:, b * 168:(b + 1) * 168], in_=ps2[:],
                                func=mybir.ActivationFunctionType.Copy)

                    # ------------ P3 tail: hole-mask ls2 + slab writeback
                    if "3" in PHASES:
                        h2ls = h2st[:].rearrange("p (t c) -> p t c", c=168)
                        nc.vector.tensor_tensor(
                            out=h2ls[:, :, 160:164], in0=h2ls[:, :, 160:164],
                            in1=nmask_sb[:].rearrange("p (t h) -> p t h", h=HEADS),
                            op=mybir.AluOpType.add)
                        nc.sync.dma_start(
                            out=h2slab[:, 0:168].rearrange("(t p) c -> p t c", p=BLK),
                            in_=h2st[:].rearrange("p (t c) -> p t c", c=168))

                # ---------------- AllGather h2 slabs
                if "4" in PHASES or "h" in PHASES:
                    nc.gpsimd.collective_compute(
                        "AllGather", mybir.AluOpType.bypass,
                        replica_groups=[list(range(NCORES))],
                        ins=[h2slab.opt()], outs=[h2full.opt()])

                # ---------------- P4: layer-2 aggregation -> out2
                p4_gather = "4" in PHASES or "7" in PHASES
                p4_comp = "4" in PHASES or "8" in PHASES
                with (
                    tc.tile_pool(name="p4g", bufs=3) as gpool,
                    tc.tile_pool(name="p4m", bufs=2) as mpool,
                    tc.tile_pool(name="p4s", bufs=3) as spool,
                    tc.tile_pool(name="p4o", bufs=1) as opool,
                ):
                    if p4_comp:
                        outst = opool.tile([BLK, NBLK * F2], f16, tag="outst")
                    for b in range(NB_RUN if (p4_gather or p4_comp) else 0):
                        wl, wh = int(Wlo[b]), int(Whi[b])
                        wt = wl + wh
                        o = int(offs[b])
                        if p4_gather:
                            G = gpool.tile([BLK, wt * TAB2], f16, tag="G2")
                            G3 = G[:].rearrange("p (w f) -> p w f", f=TAB2)
                            for (wbase, wlen, tab) in [(0, wl, h2full[0:HALF, :]),
                                                       (wl, wh, h2full[HALF:NID, :])]:
                                for w0 in range(0, wlen, GCHUNK):
                                    wn = min(GCHUNK, wlen - w0)
                                    nc.gpsimd.dma_gather(
                                        G3[:, wbase + w0:wbase + w0 + wn, :], tab,
                                        idxw_sb[:, (o + wbase + w0) * 8:(o + wbase + w0 + wn) * 8],
                                        wn * BLK, wn * BLK, TAB2,
                                        queue_num=gq[0] % NQ)
                                    gq[0] += 1
                        if not p4_comp:
                            continue
                        if not ("4" in PHASES):
                            G3 = Ggarb2[:, 0:wt * TAB2].rearrange(
                                "p (w f) -> p w f", f=TAB2)
                        lst = spool.tile([BLK, wt * HEADS], f32, tag="lst2")
                        lst3 = lst[:].rearrange("p (w h) -> p w h", h=HEADS)
                        nc.vector.tensor_tensor(
                            out=lst3, in0=G3[:, :, F2:F2 + HEADS],
                            in1=ld2sb[:, b * HEADS:(b + 1) * HEADS]
                                .unsqueeze(1).to_broadcast([BLK, wt, HEADS]),
                            op=mybir.AluOpType.add)
                        nc.vector.scalar_tensor_tensor(
                            out=lst[:], in0=lst[:], scalar=NEG_SLOPE, in1=lst[:],
                            op0=mybir.AluOpType.mult, op1=mybir.AluOpType.max)
                        p16u = spool.tile([BLK, wt * HEADS], f16, tag="p16u2")
                        p16u3 = p16u[:].rearrange("p (w h) -> p w h", h=HEADS)
                        den = spool.tile([BLK, HEADS], f32, tag="den2")
                        for h in range(HEADS):
                            nc.scalar.activation(out=p16u3[:, :, h], in_=lst3[:, :, h],
                                                 func=mybir.ActivationFunctionType.Exp,
                                                 bias=kb2[:, 0:1],
                                                 accum_out=den[:, h:h + 1])
                        nc.vector.tensor_scalar_add(out=den[:], in0=den[:], scalar1=1e-30)
                        rcp = spool.tile([BLK, HEADS], f32, tag="rcp2")
                        nc.vector.reciprocal(out=rcp[:], in_=den[:])
                        pn2 = spool.tile([BLK, wt * HEADS * 2], f16, tag="pn2b")
                        nc.vector.tensor_tensor(
                            out=pn2[:].rearrange("p (w h two) -> p w h two",
                                                 h=HEADS, two=2),
                            in0=p16u3.unsqueeze(3).to_broadcast([BLK, wt, HEADS, 2]),
                            in1=rcp[:].unsqueeze(1).unsqueeze(3)
                                .to_broadcast([BLK, wt, HEADS, 2]),
                            op=mybir.AluOpType.mult)
                        M = mpool.tile([BLK, wt * F2], f16, tag="M2")
                        C2 = N_CLS // 2
                        nc.vector.tensor_tensor(
                            out=M[:].rearrange("p (w h c2 two) -> p w h c2 two",
                                               h=HEADS, c2=C2, two=2),
                            in0=G3[:, :, 0:F2].rearrange(
                                "p w (h c2 two) -> p w h c2 two", c2=C2, two=2),
                            in1=pn2[:].rearrange("p (w h two) -> p w h two",
                                                 h=HEADS, two=2)
                                .unsqueeze(3).to_broadcast([BLK, wt, HEADS, C2, 2]),
                            op=mybir.AluOpType.mult)
                        M3 = M[:].rearrange("p (w f) -> p w f", f=F2)
                        w = wt
                        while w > 1:
                            hsz = w // 2
                            nc.vector.tensor_tensor(out=M3[:, 0:hsz, :], in0=M3[:, 0:hsz, :],
                                                    in1=M3[:, w - hsz:w, :],
                                                    op=mybir.AluOpType.add)
                            w -= hsz
                        nc.vector.tensor_copy(out=outst[:, b * F2:(b + 1) * F2],
                                              in_=M3[:, 0, :])
                    if p4_comp:
                        nc.sync.dma_start(
                            out=t_out[:].rearrange("(b p) c -> p b c", p=BLK),
                            in_=outst[:].rearrange("p (b c) -> p b c", c=F2))

    nc.compile()
    return nc


_CACHE = {}
_PREP_CACHE = {}


def _prep_cached(edge_index):
    ei = np.asarray(edge_index)
    key = (ei.shape, ei.dtype.str,
           hashlib.md5(np.ascontiguousarray(ei[:, ::101]).tobytes()).hexdigest())
    if key not in _PREP_CACHE:
        _PREP_CACHE[key] = _prep_indices(ei)
    return _PREP_CACHE[key]


def kernel(**inputs) -> np.ndarray:
    x = np.asarray(inputs["x"], np.float32)
    P = _prep_cached(inputs["edge_index"])
    node_cid = P["node_cid"]

    g = np.asarray(inputs["bn_gamma"], np.float32)
    bta = np.asarray(inputs["bn_beta"], np.float32)
    mu = np.asarray(inputs["bn_mean"], np.float32)
    var = np.asarray(inputs["bn_var"], np.float32)
    W1 = np.asarray(inputs["W1"], np.float32)
    a1s = np.asarray(inputs["a1_src"], np.float32)
    a1d = np.asarray(inputs["a1_dst"], np.float32)
    b1 = np.asarray(inputs["b1"], np.float32)
    W2 = np.asarray(inputs["W2"], np.float32)
    a2s = np.asarray(inputs["a2_src"], np.float32)
    a2d = np.asarray(inputs["a2_dst"], np.float32)

    s = g / np.sqrt(var + BN_EPS)
    W1p = (s[:, None] * W1).astype(np.float32)
    b1p = ((bta - mu * s) @ W1 + b1).astype(np.float32)
    A1s = np.zeros((HEADS * HID, HEADS), np.float32)
    A1d = np.zeros((HEADS * HID, HEADS), np.float32)
    A2s = np.zeros((HEADS * N_CLS, HEADS), np.float32)
    A2d = np.zeros((HEADS * N_CLS, HEADS), np.float32)
    for h in range(HEADS):
        A1s[h * HID:(h + 1) * HID, h] = a1s[h]
        A1d[h * HID:(h + 1) * HID, h] = a1d[h]
        A2s[h * N_CLS:(h + 1) * N_CLS, h] = a2s[h]
        A2d[h * N_CLS:(h + 1) * N_CLS, h] = a2d[h]
    W1f = np.concatenate([W1p, W1p @ A1s, W1p @ A1d], axis=1)  # [129, 136]
    csd = (b1p @ A1s + b1p @ A1d).astype(np.float32)
    W2f = np.concatenate([W2, W2 @ A2s, W2 @ A2d], axis=1).astype(np.float32)

    # x in cid space
    xp = np.zeros((NID, IN_F), np.float32)
    xp[node_cid] = x

    key = (tuple(P["Wlo"]), tuple(P["Whi"]),
           os.environ.get("GAT_PHASES", "1234"), os.environ.get("GAT_NBLK", ""),
           os.environ.get("GAT_REPEAT", ""), GCHUNK,
           os.environ.get("GAT_NQUEUES", ""), os.environ.get("GAT_TABD", ""))
    if key not in _CACHE:
        _CACHE[key] = _build_program(P["Wlo"], P["Whi"], P["offs"], P["S"])
    nc = _CACHE[key]

    common = {
        "W1f": np.ascontiguousarray(W1f[:BLK]).astype(np.float16),
        "W1row": W1f[BLK:BLK + 1].astype(np.float16),
        "W2f": W2f.astype(np.float16),
        "b1pb": np.tile(b1p[None, :], (BLK, 1)),
        "csdb": np.tile(csd[None, :], (BLK, 1)),
    }

    in_maps = []
    for q in range(NCORES):
        m = dict(common)
        qs = slice(q * SLAB, (q + 1) * SLAB)
        m["xTs"] = np.ascontiguousarray(xp[qs, :BLK].T).astype(np.float16)
        m["xls"] = np.ascontiguousarray(xp[qs, BLK:BLK + 1].T).astype(np.float16)
        m["idxw"] = np.ascontiguousarray(P["idxw"][q])
        m["nmask"] = np.ascontiguousarray(P["nmask"][q])
        in_maps.append(m)

    t0 = time.time()
    res = run_bass_kernel_spmd(nc, in_maps, core_ids=list(range(NCORES)))
    global last_run_seconds
    last_run_seconds = time.time() - t0
    outfull = np.concatenate([r["out2"] for r in res.results], axis=0)
    return outfull[node_cid].astype(np.float32)


last_run_seconds = None



# revision 25
# speedup vs baseline: 1.2195x; 1.0865x over previous
"""Trainium2 Bass kernel for the 2-layer GAT (nn_GAT_47459388621602).

Strategy (8 NeuronCores, SPMD, one NEFF; edge/graph-parallel per the
sharding hint — destination nodes sharded across cores, node-feature tables
replicated via AllGather):
  - Host: add self-loops, assign destination nodes to cores
    (degree-stratified, lo/hi-source-balanced), build per-core padded CSR
    gather index lists (int16, table split in two halves so indices fit
    int16 for dma_gather). Host prep is ~0.6s (list-based greedy balance +
    fully vectorized CSR fill), cached by an edge_index content hash.
  - Device, per core (fp16 tables/staging, f32 logit/softmax math):
      P1: fused [h1|ls1|ld1] = bn(x) @ W1[f] for the core's OWN 6272-node
          slab only (BN folded into weights on host; feature 129 via a
          second 1-partition accumulating matmul; attention projections
          a_src/a_dst folded into extra W1 columns). One whole-slab input
          DMA; one strided whole-slab table writeback.
      AllGather h1 slabs -> full fp16 table [50176, 256] (512B rows:
      [h1(128)|ls1(4)|ld1(4)|pad] — transfers under 512B pay a 2x DMA
      penalty, so the pad is free and ls1/ld1 ride along).
      P2: per 128-destination block: dma_gather source rows (<=1024 idxs
          per call — larger overflows the 1024-desc SWDGE ring), logits
          from gathered ls1 + local ld1, LeakyReLU as one fused
          scalar_tensor_tensor, K-shift exp on ACT straight to fp16 with
          per-head accumulated denominators, normalize+duplicate weights
          into adjacent pairs in one op (paired views keep innermost
          stride-1 so the big multiply runs in 2x DVE mode), fp16 tree
          reduction, +bias, ELU as 1 ACT + 2 fused DVE ops -> x2 kept
          transposed in SBUF (fp16). Pad slots gather a designated hole
          row whose ls columns are force-set to -3e4 in P1/P3, so no
          per-block mask op is needed. The layer-2 projection (P3) is
          fused into this loop per block (PE matmul + ACT copy, free).
      P3 tail: hole-mask ls2 + staged slab writeback in one DMA.
      AllGather h2 slabs -> full fp16 table [50176, 256].
      P4: layer-2 aggregation (ls2/ld2 ride in the gathered row) -> out
          slab [6272, 160] fp16, staged and written in one DMA.
  - Host re-assembles/unpermutes the 8 slabs into the full [50000, 160] f32.

Env knobs (measurement only; defaults are the production path):
  GAT_PHASES=1234, GAT_NBLK, GAT_REPEAT (loop the body inside one NEFF for
  repeat-amplified timing), GAT_GCHUNK=8.
"""
import hashlib
import os
import time

import numpy as np

import concourse.bacc as bacc
import concourse.mybir as mybir
import concourse.tile as tile
from concourse.bass_utils import run_bass_kernel_spmd
from concourse.library_config import mlp as mlp_library
from concourse.masks import make_identity

N_NODES = 50000
IN_F = 129
HID = 32
HEADS = 4
N_CLS = 40
NEG_SLOPE = 0.2
BN_EPS = 1e-5
NCORES = 8
BLK = 128
NBLK = 49
SLAB = NBLK * BLK           # 6272
NID = NCORES * SLAB         # 50176
HALF = NID // 2             # 25088
NEGM = -30000.0
K1 = 8.0
K2 = 12.0
F2 = 160                    # layer-2 message width
TAB1 = 256                  # fp16 layer-1 gather row (512B): [h1|ls1|ld1|pad]
TAB2 = 256                  # fp16 layer-2 gather row (512B): [h2|ls2|ld2|pad]
GCHUNK = int(os.environ.get("GAT_GCHUNK", "8"))  # w-cols (x128 idxs) per dma_gather
GBUFS = int(os.environ.get("GAT_GBUFS", "3"))    # gather-tile double-buffer depth
GRP = 4 * BLK

f32 = mybir.dt.float32
f16 = mybir.dt.float16
i16 = mybir.dt.int16


# ----------------------------------------------------------------- host prep
def _balance_greedy(src, dst):
    order = np.argsort(src, kind="stable")
    dst_by_src = dst[order]
    s_starts = np.searchsorted(src[order], np.arange(N_NODES))
    s_ends = np.searchsorted(src[order], np.arange(N_NODES) + 1)
    outdeg = s_ends - s_starts
    balance = [0] * N_NODES
    is_lo = np.zeros(N_NODES, dtype=bool)
    dbl = dst_by_src.tolist()
    cap = N_NODES // 2
    n_lo = n_hi = 0
    for n in np.argsort(-outdeg, kind="stable").tolist():
        s0, s1 = s_starts[n], s_ends[n]
        nb = dbl[s0:s1]
        tot = 0
        for d in nb:
            tot += balance[d]
        go_lo = tot <= 0
        if go_lo and n_lo >= cap:
            go_lo = False
        if (not go_lo) and n_hi >= cap:
            go_lo = True
        if go_lo:
            is_lo[n] = True
            n_lo += 1
            for d in nb:
                balance[d] += 1
        else:
            n_hi += 1
            for d in nb:
                balance[d] -= 1
    return is_lo


def _prep_indices(edge_index):
    src0 = np.asarray(edge_index[0], dtype=np.int64)
    dst0 = np.asarray(edge_index[1], dtype=np.int64)
    loops = np.arange(N_NODES, dtype=np.int64)
    src = np.concatenate([src0, loops])
    dst = np.concatenate([dst0, loops])
    deg = np.bincount(dst, minlength=N_NODES)

    is_lo_node = _balance_greedy(src, dst)
    is_lo_src = is_lo_node[src]
    deglo = np.bincount(dst[is_lo_src], minlength=N_NODES)
    deghi = deg - deglo

    # degree-stratified assignment; residue slot order keeps chunk types
    # aligned across cores so slot-wise max W is tight
    blocks = {}
    for half in range(2):
        ids = np.where(is_lo_node if half == 0 else ~is_lo_node)[0]
        ids = ids[np.argsort(-deg[ids], kind="stable")]
        n_strata = (len(ids) + GRP - 1) // GRP
        assert n_strata <= NBLK
        core_blocks = [[] for _ in range(4)]
        for s in range(n_strata):
            members = ids[s * GRP: min((s + 1) * GRP, len(ids))]
            m_sorted = members[np.argsort(-deglo[members], kind="stable")]
            chs = np.array_split(m_sorted, 4)
            for t, ch in enumerate(chs):
                core_blocks[(t - s) % 4].append((s, ch))
        for q in range(4):
            core_blocks[q].sort(key=lambda x: (x[0] // 4) * 4 + (x[0] + q) % 4)
            for b in range(NBLK):
                ch = core_blocks[q][b][1] if b < len(core_blocks[q]) else np.array([], dtype=np.int64)
                blk = ch[np.argsort(-deglo[ch], kind="stable")] if len(ch) else ch
                blocks[(half * 4 + q, b)] = blk

    node_cid = np.empty(N_NODES, dtype=np.int64)
    Wlo_qb = np.ones((NCORES, NBLK), dtype=np.int64)
    Whi_qb = np.ones((NCORES, NBLK), dtype=np.int64)
    for q in range(NCORES):
        for b in range(NBLK):
            blk = blocks[(q, b)]
            node_cid[blk] = q * SLAB + b * BLK + np.arange(len(blk))
            if len(blk):
                Wlo_qb[q, b] = max(1, int(deglo[blk].max()))
                Whi_qb[q, b] = max(1, int(deghi[blk].max()))
    Wlo = Wlo_qb.max(axis=0)
    Whi = Whi_qb.max(axis=0)
    S = int((Wlo + Whi).sum())
    offs = np.zeros(NBLK + 1, dtype=np.int64)
    offs[1:] = np.cumsum(Wlo + Whi)

    # pad slots gather a "hole" row (an unused cid) whose ls columns are
    # force-written to NEGM on device, so exp()->0 without any mask op
    holes = np.setdiff1d(np.arange(NID), node_cid)
    hole_lo = int(holes[holes < HALF][0])
    hole_hi = int(holes[holes >= HALF][0])

    # vectorized padded-CSR fill
    E = len(src)
    src_cid = node_cid[src]
    dst_cid = node_cid[dst]
    eorder = np.argsort(dst_cid, kind="stable")
    sc = src_cid[eorder]
    dc = dst_cid[eorder]
    lo = is_lo_src[eorder]
    starts = np.searchsorted(dc, np.arange(NID))
    lo_cum = np.concatenate([[0], np.cumsum(lo)])
    pos = np.arange(E)
    seg_start = starts[dc]
    lo_rank = lo_cum[pos] - lo_cum[seg_start]
    hi_rank = (pos - seg_start) - lo_rank
    b_of = (dc // BLK) % NBLK
    q_of = dc // SLAB
    jj_of = dc % BLK
    col = offs[b_of] + np.where(lo, lo_rank, Wlo[b_of] + hi_rank)
    val = np.where(lo, sc, sc - HALF).astype(np.int16)
    flat = (q_of * BLK + jj_of) * S + col
    colhi = np.zeros(S, dtype=bool)
    for b in range(NBLK):
        colhi[offs[b] + Wlo[b]:offs[b + 1]] = True
    idx16 = np.where(colhi, np.int16(hole_hi - HALF),
                     np.int16(hole_lo))[None, None, :].repeat(
                         NCORES, 0).repeat(BLK, 1).astype(np.int16)
    idx16.ravel()[flat] = val
    # per-(core, block) fill levels -> hole-row mask [NCORES, BLK, NBLK]
    lens = np.zeros((NCORES, NBLK), dtype=np.int64)
    for q in range(NCORES):
        for b in range(NBLK):
            lens[q, b] = len(blocks[(q, b)])
    jj = np.arange(BLK)
    nmask = np.where(jj[None, :, None] < lens[:, None, :],
                     np.float16(0.0), np.float16(NEGM))  # [NCORES, BLK, NBLK]
    nmask = np.repeat(nmask[:, :, :, None], HEADS, axis=3).reshape(
        NCORES, BLK, NBLK * HEADS)

    # compact wrapped idx stream [NCORES, 16, S*8]; device replicates to 128
    idxw = np.zeros((NCORES, 16, S * 8), dtype=np.int16)
    col0 = 0
    for b in range(NBLK):
        o = int(offs[b])
        for (w0, w1) in ((0, int(Wlo[b])), (int(Wlo[b]), int(Wlo[b] + Whi[b]))):
            nw = w1 - w0
            sl = idx16[:, :, o + w0:o + w1].transpose(0, 2, 1).reshape(NCORES, nw * BLK)
            idxw[:, :, col0:col0 + nw * 8] = \
                sl.reshape(NCORES, nw * 8, 16).transpose(0, 2, 1)
            col0 += nw * 8
    assert col0 == S * 8

    return dict(node_cid=node_cid, Wlo=Wlo.astype(int), Whi=Whi.astype(int),
                offs=offs, S=S, idxw=idxw, nmask=nmask)


# ----------------------------------------------------------------- program
def _build_program(Wlo, Whi, offs, S):
    PHASES = os.environ.get("GAT_PHASES", "1234")
    NB_RUN = int(os.environ.get("GAT_NBLK", str(NBLK)))
    REPEAT = int(os.environ.get("GAT_REPEAT", "1"))
    NQ = int(os.environ.get("GAT_NQUEUES", "4"))
    SCRATCH = int(os.environ.get("GAT_SCRATCH", "16384"))
    nc = bacc.Bacc("TRN2", target_bir_lowering=False, debug=False,
                   num_devices=NCORES, num_swdge_queues=NQ,
                   dynamic_dma_scratch_size=SCRATCH)
    gq = [0]

    # inputs
    t_xTs = nc.dram_tensor("xTs", [BLK, SLAB], f16, kind="ExternalInput")
    t_xls = nc.dram_tensor("xls", [1, SLAB], f16, kind="ExternalInput")
    t_W1 = nc.dram_tensor("W1f", [BLK, 136], f16, kind="ExternalInput")
    t_W1r = nc.dram_tensor("W1row", [1, 136], f16, kind="ExternalInput")
    t_W2 = nc.dram_tensor("W2f", [BLK, 168], f16, kind="ExternalInput")
    t_b1p = nc.dram_tensor("b1pb", [BLK, BLK], f32, kind="ExternalInput")
    t_csd = nc.dram_tensor("csdb", [BLK, HEADS], f32, kind="ExternalInput")
    t_idxw = nc.dram_tensor("idxw", [16, S * 8], i16, kind="ExternalInput")
    t_nmask = nc.dram_tensor("nmask", [BLK, NBLK * HEADS], f16,
                             kind="ExternalInput")
    t_out = nc.dram_tensor("out2", [SLAB, F2], f16, kind="ExternalOutput")

    with tile.TileContext(nc) as tc:
        with (
            tc.tile_pool(name="const", bufs=1) as cpool,
            tc.tile_pool(name="dram", bufs=1, space="DRAM") as dpool,
        ):
            nc.gpsimd.load_library(mlp_library)

            # resident constants
            W1sb = cpool.tile([BLK, 136], f16)
            nc.sync.dma_start(out=W1sb[:], in_=t_W1[:])
            W1rsb = cpool.tile([1, 136], f16)
            nc.sync.dma_start(out=W1rsb[:], in_=t_W1r[:])
            W2sb = cpool.tile([BLK, 168], f16)
            nc.sync.dma_start(out=W2sb[:], in_=t_W2[:])
            b1pb = cpool.tile([BLK, BLK], f32)
            nc.sync.dma_start(out=b1pb[:], in_=t_b1p[:])
            csdb = cpool.tile([BLK, HEADS], f32)
            nc.sync.dma_start(out=csdb[:], in_=t_csd[:])
            idxw_sb = cpool.tile([BLK, S * 8], i16)
            for k in range(8):
                nc.sync.dma_start(out=idxw_sb[16 * k:16 * (k + 1), :],
                                  in_=t_idxw[:])
            nmask_sb = cpool.tile([BLK, NBLK * HEADS], f16)
            nc.sync.dma_start(out=nmask_sb[:], in_=t_nmask[:])
            ident = cpool.tile([BLK, BLK], f32)
            make_identity(nc, ident[:])
            kb1 = cpool.tile([BLK, 1], f32)
            nc.vector.memset(kb1[:], -K1)
            kb2 = cpool.tile([BLK, 1], f32)
            nc.vector.memset(kb2[:], -K2)
            zeros = cpool.tile([BLK, BLK], f32)
            nc.vector.memset(zeros[:], 0.0)
            # persistent per-slab state
            ld1sb = cpool.tile([BLK, NBLK * HEADS], f32)
            ld2sb = cpool.tile([BLK, NBLK * HEADS], f32)
            x2Tsb = cpool.tile([BLK, SLAB], f16)
            # diagnostic compute-only modes: resident garbage gather tile
            wtmax = int(max(Wlo[b] + Whi[b] for b in range(NBLK)))
            if "6" in PHASES and "2" not in PHASES:
                Ggarb = cpool.tile([BLK, wtmax * TAB1], f16)
                nc.vector.memset(Ggarb[:], 0.5)
            if "8" in PHASES and "4" not in PHASES:
                Ggarb2 = cpool.tile([BLK, wtmax * TAB2], f16)
                nc.vector.memset(Ggarb2[:], 0.5)

            for _rep in range(REPEAT):
                # internal DRAM (per repetition: Shared tensors allow only a
                # single writing instruction)
                h1slab = dpool.tile([SLAB, TAB1], f16)
                h1full = dpool.tile([NID, TAB1], f16, addr_space="Shared")
                h2slab = dpool.tile([SLAB, TAB2], f16)
                h2full = dpool.tile([NID, TAB2], f16, addr_space="Shared")

                # ---------------- P1: own-slab h1 / ld1 (batched DMA)
                with (
                    tc.tile_pool(name="p1", bufs=1) as pool,
                    tc.tile_pool(name="p1ps", bufs=2, space="PSUM") as pspool,
                ):
                    if "1" in PHASES:
                        xTs_sb = pool.tile([BLK, SLAB], f16, tag="xTs")
                        nc.sync.dma_start(out=xTs_sb[:], in_=t_xTs[:])
                        xls_sb = pool.tile([1, SLAB], f16, tag="xls")
                        nc.sync.dma_start(out=xls_sb[:], in_=t_xls[:])
                        h1st = pool.tile([BLK, NBLK * 136], f16, tag="h1st")
                    for t in range(NBLK if "1" in PHASES else 0):
                        sl = slice(t * BLK, (t + 1) * BLK)
                        ps = pspool.tile([BLK, 136], f32)
                        nc.tensor.matmul(out=ps[:], lhsT=xTs_sb[:, sl], rhs=W1sb[:],
                                         start=True, stop=False)
                        nc.tensor.matmul(out=ps[:], lhsT=xls_sb[:, sl], rhs=W1rsb[:],
                                         start=False, stop=True)
                        nc.vector.tensor_copy(out=ld1sb[:, t * HEADS:(t + 1) * HEADS],
                                              in_=ps[:, 132:136])
                        nc.vector.tensor_copy(out=h1st[:, t * 136:(t + 1) * 136],
                                              in_=ps[:])
                    if "1" in PHASES:
                        # hole rows: force ls columns to NEGM so any gather of
                        # them yields exp()->0 (replaces the per-block mask op)
                        h1ls = h1st[:].rearrange("p (t c) -> p t c", c=136)
                        nc.vector.tensor_tensor(
                            out=h1ls[:, :, 128:132], in0=h1ls[:, :, 128:132],
                            in1=nmask_sb[:].rearrange("p (t h) -> p t h", h=HEADS),
                            op=mybir.AluOpType.add)
                        nc.sync.dma_start(
                            out=h1slab[:, 0:136].rearrange("(t p) c -> p t c", p=BLK),
                            in_=h1st[:].rearrange("p (t c) -> p t c", c=136))

                # ---------------- AllGather h1 slabs
                if "2" in PHASES or "g" in PHASES:
                    nc.gpsimd.collective_compute(
                        "AllGather", mybir.AluOpType.bypass,
                        replica_groups=[list(range(NCORES))],
                        ins=[h1slab.opt()], outs=[h1full.opt()])

                # ---------------- P2: layer-1 aggregation -> x2Tsb (SBUF, fp16)
                p2_gather = "2" in PHASES or "5" in PHASES
                p2_comp = "2" in PHASES or "6" in PHASES
                with (
                    tc.tile_pool(name="p2g", bufs=GBUFS) as gpool,
                    tc.tile_pool(name="p2m", bufs=2) as mpool,
                    tc.tile_pool(name="p2s", bufs=3) as spool,
                    tc.tile_pool(name="p2h", bufs=1) as h2pool,
                    tc.tile_pool(name="p2ps", bufs=2, space="PSUM") as pspool,
                ):
                    if "3" in PHASES:
                        h2st = h2pool.tile([BLK, NBLK * 168], f16, tag="h2st")
                    for b in range(NB_RUN if (p2_gather or p2_comp) else 0):
                        wl, wh = int(Wlo[b]), int(Whi[b])
                        wt = wl + wh
                        o = int(offs[b])
                        if p2_gather:
                            # diag knob: gather a smaller slice of each row
                            # (bytes/descriptor probe; only for gather-only mode)
                            TABd = int(os.environ.get("GAT_TABD", "0"))
                            TABg = TABd if (TABd and "2" not in PHASES) else TAB1
                            G = gpool.tile([BLK, wt * TABg], f16, tag="G")
                            G3 = G[:].rearrange("p (w f) -> p w f", f=TABg)
                            for (wbase, wlen, tab) in [(0, wl, h1full[0:HALF, 0:TABg]),
                                                       (wl, wh, h1full[HALF:NID, 0:TABg])]:
                                for w0 in range(0, wlen, GCHUNK):
                                    wn = min(GCHUNK, wlen - w0)
                                    nc.gpsimd.dma_gather(
                                        G3[:, wbase + w0:wbase + w0 + wn, :], tab,
                                        idxw_sb[:, (o + wbase + w0) * 8:(o + wbase + w0 + wn) * 8],
                                        wn * BLK, wn * BLK, TABg,
                                        elem_step=TAB1,
                                        queue_num=gq[0] % NQ)
                                    gq[0] += 1
                        if not p2_comp:
                            continue
                        if not ("2" in PHASES):
                            G3 = Ggarb[:, 0:wt * TAB1].rearrange(
                                "p (w f) -> p w f", f=TAB1)
                        ldc = spool.tile([BLK, HEADS], f32, tag="ldc")
                        nc.vector.tensor_tensor(
                            out=ldc[:], in0=ld1sb[:, b * HEADS:(b + 1) * HEADS],
                            in1=csdb[:], op=mybir.AluOpType.add)
                        M = mpool.tile([BLK, wt * BLK], f16, tag="M")
                        lst = spool.tile([BLK, wt * HEADS], f32, tag="lst")
                        lst3 = lst[:].rearrange("p (w h) -> p w h", h=HEADS)
                        nc.vector.tensor_tensor(
                            out=lst3, in0=G3[:, :, BLK:BLK + HEADS],
                            in1=ldc[:].unsqueeze(1).to_broadcast([BLK, wt, HEADS]),
                            op=mybir.AluOpType.add)
                        # leaky relu in one pass: max(x, slope*x)
                        nc.vector.scalar_tensor_tensor(
                            out=lst[:], in0=lst[:], scalar=NEG_SLOPE, in1=lst[:],
                            op0=mybir.AluOpType.mult, op1=mybir.AluOpType.max)
                        # exp (K-shifted) directly to fp16, accumulate denoms
                        p16u = spool.tile([BLK, wt * HEADS], f16, tag="p16u")
                        p16u3 = p16u[:].rearrange("p (w h) -> p w h", h=HEADS)
                        den = spool.tile([BLK, HEADS], f32, tag="den")
                        for h in range(HEADS):
                            nc.scalar.activation(out=p16u3[:, :, h], in_=lst3[:, :, h],
                                                 func=mybir.ActivationFunctionType.Exp,
                                                 bias=kb1[:, 0:1],
                                                 accum_out=den[:, h:h + 1])
                        nc.vector.tensor_scalar_add(out=den[:], in0=den[:], scalar1=1e-30)
                        rcp = spool.tile([BLK, HEADS], f32, tag="rcp")
                        nc.vector.reciprocal(out=rcp[:], in_=den[:])
                        # normalize + duplicate each weight (pairs enable 2x TT)
                        pn2 = spool.tile([BLK, wt * HEADS * 2], f16, tag="pn2")
                        nc.vector.tensor_tensor(
                            out=pn2[:].rearrange("p (w h two) -> p w h two",
                                                 h=HEADS, two=2),
                            in0=p16u3.unsqueeze(3).to_broadcast([BLK, wt, HEADS, 2]),
                            in1=rcp[:].unsqueeze(1).unsqueeze(3)
                                .to_broadcast([BLK, wt, HEADS, 2]),
                            op=mybir.AluOpType.mult)
                        # M = G * pn (paired views keep innermost stride-1: 2x DVE)
                        C2 = HID // 2
                        nc.vector.tensor_tensor(
                            out=M[:].rearrange("p (w h c2 two) -> p w h c2 two",
                                               h=HEADS, c2=C2, two=2),
                            in0=G3[:, :, 0:BLK].rearrange(
                                "p w (h c2 two) -> p w h c2 two", c2=C2, two=2),
                            in1=pn2[:].rearrange("p (w h two) -> p w h two",
                                                 h=HEADS, two=2)
                                .unsqueeze(3).to_broadcast([BLK, wt, HEADS, C2, 2]),
                            op=mybir.AluOpType.mult)
                        # tree reduce over w (fp16)
                        M3 = M[:].rearrange("p (w f) -> p w f", f=BLK)
                        w = wt
                        while w > 1:
                            hsz = w // 2
                            nc.vector.tensor_tensor(out=M3[:, 0:hsz, :], in0=M3[:, 0:hsz, :],
                                                    in1=M3[:, w - hsz:w, :],
                                                    op=mybir.AluOpType.add)
                            w -= hsz
                        x2 = spool.tile([BLK, BLK], f32, tag="x2")
                        nc.vector.tensor_tensor(out=x2[:], in0=M3[:, 0, :], in1=b1pb[:],
                                                op=mybir.AluOpType.add)
                        # elu: x2 = max(x2,0) + min(exp(x2)-1, 0)  (1 ACT + 2 fused DVE)
                        ex = spool.tile([BLK, BLK], f32, tag="ex")
                        nc.scalar.activation(out=ex[:], in_=x2[:],
                                             func=mybir.ActivationFunctionType.Exp)
                        nc.vector.scalar_tensor_tensor(
                            out=ex[:], in0=ex[:], scalar=-1.0, in1=zeros[:],
                            op0=mybir.AluOpType.add, op1=mybir.AluOpType.min)
                        nc.vector.scalar_tensor_tensor(
                            out=x2[:], in0=x2[:], scalar=0.0, in1=ex[:],
                            op0=mybir.AluOpType.max, op1=mybir.AluOpType.add)
                        # transpose -> persistent SBUF slab (fp16)
                        tps = pspool.tile([BLK, BLK], f32, tag="tps")
                        nc.tensor.transpose(out=tps[:], in_=x2[:], identity=ident[:])
                        nc.vector.tensor_copy(out=x2Tsb[:, b * BLK:(b + 1) * BLK],
                                              in_=tps[:])
                        # fused P3: layer-2 projection for this block (PE + ACT)
                        if "3" in PHASES:
                            ps2 = pspool.tile([BLK, 168], f32, tag="mm2")
                            nc.tensor.matmul(
                                out=ps2[:], lhsT=x2Tsb[:, b * BLK:(b + 1) * BLK],
                                rhs=W2sb[:], start=True, stop=True)
                            nc.vector.tensor_copy(
                                out=ld2sb[:, b * HEADS:(b + 1) * HEADS],
                                in_=ps2[:, 164:168])
                            nc.scalar.activation(
                                out=h2st[:, b * 168:(b + 1) * 168], in_=ps2[:],
                                func=mybir.ActivationFunctionType.Copy)

                    # ------------ P3 tail: hole-mask ls2 + slab writeback
                    if "3" in PHASES:
                        h2ls = h2st[:].rearrange("p (t c) -> p t c", c=168)
                        nc.vector.tensor_tensor(
                            out=h2ls[:, :, 160:164], in0=h2ls[:, :, 160:164],
                            in1=nmask_sb[:].rearrange("p (t h) -> p t h", h=HEADS),
                            op=mybir.AluOpType.add)
                        nc.sync.dma_start(
                            out=h2slab[:, 0:168].rearrange("(t p) c -> p t c", p=BLK),
                            in_=h2st[:].rearrange("p (t c) -> p t c", c=168))

                # ---------------- AllGather h2 slabs
                if "4" in PHASES or "h" in PHASES:
                    nc.gpsimd.collective_compute(
                        "AllGather", mybir.AluOpType.bypass,
                        replica_groups=[list(range(NCORES))],
                        ins=[h2slab.opt()], outs=[h2full.opt()])

                # ---------------- P4: layer-2 aggregation -> out2
                p4_gather = "4" in PHASES or "7" in PHASES
                p4_comp = "4" in PHASES or "8" in PHASES
                with (
                    tc.tile_pool(name="p4g", bufs=GBUFS) as gpool,
                    tc.tile_pool(name="p4m", bufs=2) as mpool,
                    tc.tile_pool(name="p4s", bufs=3) as spool,
                    tc.tile_pool(name="p4o", bufs=1) as opool,
                ):
                    if p4_comp:
                        outst = opool.tile([BLK, NBLK * F2], f16, tag="outst")
                    for b in range(NB_RUN if (p4_gather or p4_comp) else 0):
                        wl, wh = int(Wlo[b]), int(Whi[b])
                        wt = wl + wh
                        o = int(offs[b])
                        if p4_gather:
                            G = gpool.tile([BLK, wt * TAB2], f16, tag="G2")
                            G3 = G[:].rearrange("p (w f) -> p w f", f=TAB2)
                            for (wbase, wlen, tab) in [(0, wl, h2full[0:HALF, :]),
                                                       (wl, wh, h2full[HALF:NID, :])]:
                                for w0 in range(0, wlen, GCHUNK):
                                    wn = min(GCHUNK, wlen - w0)
                                    nc.gpsimd.dma_gather(
                                        G3[:, wbase + w0:wbase + w0 + wn, :], tab,
                                        idxw_sb[:, (o + wbase + w0) * 8:(o + wbase + w0 + wn) * 8],
                                        wn * BLK, wn * BLK, TAB2,
                                        queue_num=gq[0] % NQ)
                                    gq[0] += 1
                        if not p4_comp:
                            continue
                        if not ("4" in PHASES):
                            G3 = Ggarb2[:, 0:wt * TAB2].rearrange(
                                "p (w f) -> p w f", f=TAB2)
                        lst = spool.tile([BLK, wt * HEADS], f32, tag="lst2")
                        lst3 = lst[:].rearrange("p (w h) -> p w h", h=HEADS)
                        nc.vector.tensor_tensor(
                            out=lst3, in0=G3[:, :, F2:F2 + HEADS],
                            in1=ld2sb[:, b * HEADS:(b + 1) * HEADS]
                                .unsqueeze(1).to_broadcast([BLK, wt, HEADS]),
                            op=mybir.AluOpType.add)
                        nc.vector.scalar_tensor_tensor(
                            out=lst[:], in0=lst[:], scalar=NEG_SLOPE, in1=lst[:],
                            op0=mybir.AluOpType.mult, op1=mybir.AluOpType.max)
                        p16u = spool.tile([BLK, wt * HEADS], f16, tag="p16u2")
                        p16u3 = p16u[:].rearrange("p (w h) -> p w h", h=HEADS)
                        den = spool.tile([BLK, HEADS], f32, tag="den2")
                        for h in range(HEADS):
                            nc.scalar.activation(out=p16u3[:, :, h], in_=lst3[:, :, h],
                                                 func=mybir.ActivationFunctionType.Exp,
                                                 bias=kb2[:, 0:1],
                                                 accum_out=den[:, h:h + 1])
                        nc.vector.tensor_scalar_add(out=den[:], in0=den[:], scalar1=1e-30)
                        rcp = spool.tile([BLK, HEADS], f32, tag="rcp2")
                        nc.vector.reciprocal(out=rcp[:], in_=den[:])
                        pn2 = spool.tile([BLK, wt * HEADS * 2], f16, tag="pn2b")
                        nc.vector.tensor_tensor(
                            out=pn2[:].rearrange("p (w h two) -> p w h two",
                                                 h=HEADS, two=2),
                            in0=p16u3.unsqueeze(3).to_broadcast([BLK, wt, HEADS, 2]),
                            in1=rcp[:].unsqueeze(1).unsqueeze(3)
                                .to_broadcast([BLK, wt, HEADS, 2]),
                            op=mybir.AluOpType.mult)
                        M = mpool.tile([BLK, wt * F2], f16, tag="M2")
                        C2 = N_CLS // 2
                        nc.vector.tensor_tensor(
                            out=M[:].rearrange("p (w h c2 two) -> p w h c2 two",
                                               h=HEADS, c2=C2, two=2),
                            in0=G3[:, :, 0:F2].rearrange(
                                "p w (h c2 two) -> p w h c2 two", c2=C2, two=2),
                            in1=pn2[:].rearrange("p (w h two) -> p w h two",
                                                 h=HEADS, two=2)
                                .unsqueeze(3).to_broadcast([BLK, wt, HEADS, C2, 2]),
                            op=mybir.AluOpType.mult)
                        M3 = M[:].rearrange("p (w f) -> p w f", f=F2)
                        w = wt
                        while w > 1:
                            hsz = w // 2
                            nc.vector.tensor_tensor(out=M3[:, 0:hsz, :], in0=M3[:, 0:hsz, :],
                                                    in1=M3[:, w - hsz:w, :],
                                                    op=mybir.AluOpType.add)
                            w -= hsz
                        nc.vector.tensor_copy(out=outst[:, b * F2:(b + 1) * F2],
                                              in_=M3[:, 0, :])
                    if p4_comp:
                        nc.sync.dma_start(
                            out=t_out[:].rearrange("(b p) c -> p b c", p=BLK),
                            in_=outst[:].rearrange("p (b c) -> p b c", c=F2))

    nc.compile()
    return nc


_CACHE = {}
_PREP_CACHE = {}


def _prep_cached(edge_index):
    ei = np.asarray(edge_index)
    key = (ei.shape, ei.dtype.str,
           hashlib.md5(np.ascontiguousarray(ei[:, ::101]).tobytes()).hexdigest())
    if key not in _PREP_CACHE:
        _PREP_CACHE[key] = _prep_indices(ei)
    return _PREP_CACHE[key]


def kernel(**inputs) -> np.ndarray:
    x = np.asarray(inputs["x"], np.float32)
    P = _prep_cached(inputs["edge_index"])
    node_cid = P["node_cid"]

    g = np.asarray(inputs["bn_gamma"], np.float32)
    bta = np.asarray(inputs["bn_beta"], np.float32)
    mu = np.asarray(inputs["bn_mean"], np.float32)
    var = np.asarray(inputs["bn_var"], np.float32)
    W1 = np.asarray(inputs["W1"], np.float32)
    a1s = np.asarray(inputs["a1_src"], np.float32)
    a1d = np.asarray(inputs["a1_dst"], np.float32)
    b1 = np.asarray(inputs["b1"], np.float32)
    W2 = np.asarray(inputs["W2"], np.float32)
    a2s = np.asarray(inputs["a2_src"], np.float32)
    a2d = np.asarray(inputs["a2_dst"], np.float32)

    s = g / np.sqrt(var + BN_EPS)
    W1p = (s[:, None] * W1).astype(np.float32)
    b1p = ((bta - mu * s) @ W1 + b1).astype(np.float32)
    A1s = np.zeros((HEADS * HID, HEADS), np.float32)
    A1d = np.zeros((HEADS * HID, HEADS), np.float32)
    A2s = np.zeros((HEADS * N_CLS, HEADS), np.float32)
    A2d = np.zeros((HEADS * N_CLS, HEADS), np.float32)
    for h in range(HEADS):
        A1s[h * HID:(h + 1) * HID, h] = a1s[h]
        A1d[h * HID:(h + 1) * HID, h] = a1d[h]
        A2s[h * N_CLS:(h + 1) * N_CLS, h] = a2s[h]
        A2d[h * N_CLS:(h + 1) * N_CLS, h] = a2d[h]
    W1f = np.concatenate([W1p, W1p @ A1s, W1p @ A1d], axis=1)  # [129, 136]
    csd = (b1p @ A1s + b1p @ A1d).astype(np.float32)
    W2f = np.concatenate([W2, W2 @ A2s, W2 @ A2d], axis=1).astype(np.float32)

    # x in cid space
    xp = np.zeros((NID, IN_F), np.float32)
    xp[node_cid] = x

    key = (tuple(P["Wlo"]), tuple(P["Whi"]),
           os.environ.get("GAT_PHASES", "1234"), os.environ.get("GAT_NBLK", ""),
           os.environ.get("GAT_REPEAT", ""), GCHUNK,
           os.environ.get("GAT_NQUEUES", ""), os.environ.get("GAT_TABD", ""),
           GBUFS, os.environ.get("GAT_SCRATCH", ""))
    if key not in _CACHE:
        _CACHE[key] = _build_program(P["Wlo"], P["Whi"], P["offs"], P["S"])
    nc = _CACHE[key]

    common = {
        "W1f": np.ascontiguousarray(W1f[:BLK]).astype(np.float16),
        "W1row": W1f[BLK:BLK + 1].astype(np.float16),
        "W2f": W2f.astype(np.float16),
        "b1pb": np.tile(b1p[None, :], (BLK, 1)),
        "csdb": np.tile(csd[None, :], (BLK, 1)),
    }

    in_maps = []
    for q in range(NCORES):
        m = dict(common)
        qs = slice(q * SLAB, (q + 1) * SLAB)
        m["xTs"] = np.ascontiguousarray(xp[qs, :BLK].T).astype(np.float16)
        m["xls"] = np.ascontiguousarray(xp[qs, BLK:BLK + 1].T).astype(np.float16)
        m["idxw"] = np.ascontiguousarray(P["idxw"][q])
        m["nmask"] = np.ascontiguousarray(P["nmask"][q])
        in_maps.append(m)

    t0 = time.time()
    res = run_bass_kernel_spmd(nc, in_maps, core_ids=list(range(NCORES)))
    global last_run_seconds
    last_run_seconds = time.time() - t0
    outfull = np.concatenate([r["out2"] for r in res.results], axis=0)
    return outfull[node_cid].astype(np.float32)


last_run_seconds = None

